# revision 4
# baseline (speedup 1.0000x reference)
"""Causal self-attention (B=2, S=2048, E=1024, H=16, D=64) on 8 trn2 NeuronCores.

Sharding: core c = (batch b = c // 4, head-group g = c % 4).  Each core computes
4 heads (one quarter of the 16) for one batch: projections q/k/v for its 256
output channels, then causal flash-style attention, writing out[b, :, 256g:256g+256].

Per-core kernel design (Bass/Tile):
  - Host pre-transposes hidden -> hT [E, S] (bf16) and weight slices -> wT [E, 256]
    (bf16) so all matmul contractions have K on partitions.
  - q/k projections (bf16, PSUM-accumulated over 8 E-chunks) produce qT/kT in
    [d, t] layout, copied to SBUF as float32r with scale 1/8 (q) and bias add.
  - v projection produces v in [t, d] layout; DVE copy splits heads into
    v_aug tiles [tk=128, 65*2] with a ones column per head (sum-of-exp trick).
  - scores^T tiles [tk=128, tq=512] per head via single f32r matmuls; the two
    heads of a pair run concurrently on PE row halves (K=64 each).
  - exp via ScalarE activation (attention-mask bias per tk partition), bf16 out.
  - causal masking: gpsimd affine_select zeroes the invalid region of
    diagonal-crossing tiles after exp.
  - attn @ v_aug accumulates unnormalized out^T [65, tq] in PSUM (bf16 matmuls);
    row 64 is the softmax denominator.
  - PE transpose [65,128] -> [128,65], then DVE reciprocal + tensor_scalar mul
    normalizes and writes [t, d] output tiles; DMA to DRAM.
"""

import numpy as np
import ml_dtypes

import concourse.bass as bass
import concourse.mybir as mybir
import concourse.tile as tile
from concourse import bacc
from concourse.bass_utils import run_bass_kernel_spmd

F32 = mybir.dt.float32
F32R = mybir.dt.float32r
BF16 = mybir.dt.bfloat16

B, S, E = 2, 2048, 1024
H, D = 16, 64
NCORES = 8
OC = 256          # output channels per core (4 heads)
NPAIR = 2         # head pairs per core
NT = S // 128     # 16 tk tiles
NT4 = S // 512    # 4 tq blocks

_cached_nc = None


def _build():
    nc = bacc.Bacc()

    hT = nc.declare_dram_parameter("hT", [E, S], BF16, isOutput=False)
    wqT = nc.declare_dram_parameter("wqT", [E, OC], BF16, isOutput=False)
    wkT = nc.declare_dram_parameter("wkT", [E, OC], BF16, isOutput=False)
    wvT = nc.declare_dram_parameter("wvT", [E, OC], BF16, isOutput=False)
    bqp = nc.declare_dram_parameter("bqp", [128, 2], F32, isOutput=False)
    bkp = nc.declare_dram_parameter("bkp", [128, 2], F32, isOutput=False)
    bvf = nc.declare_dram_parameter("bvf", [OC], F32, isOutput=False)
    mask_t = nc.declare_dram_parameter("mask_t", [128, NT], F32, isOutput=False)
    ident = nc.declare_dram_parameter("ident", [65, 65], F32, isOutput=False)
    out = nc.declare_dram_parameter("out", [S, OC], F32, isOutput=True)

    EXP = mybir.ActivationFunctionType.Exp
    ADD = mybir.AluOpType.add
    MULT = mybir.AluOpType.mult
    GE = mybir.AluOpType.is_ge

    with tile.TileContext(nc) as tc:
        with (
            tc.tile_pool(name="cst", bufs=1) as cst,
            tc.tile_pool(name="work", bufs=3) as work,
            tc.tile_pool(name="expp", bufs=6) as expp,
            tc.tile_pool(name="ps_small", bufs=2, space="PSUM") as ps_small,
            tc.tile_pool(name="ps_sc", bufs=2, space="PSUM") as ps_sc,
            tc.tile_pool(name="ps_out", bufs=2, space="PSUM") as ps_out,
        ):
            # ---- constants ----
            mask_sb = cst.tile([128, NT], F32, tag="mask")
            nc.sync.dma_start(out=mask_sb, in_=mask_t[:, :])
            ident_sb = cst.tile([65, 65], F32, tag="ident")
            nc.sync.dma_start(out=ident_sb, in_=ident[:, :])
            bq_sb = cst.tile([128, 2], F32, tag="bq")
            nc.sync.dma_start(out=bq_sb, in_=bqp[:, :])
            bk_sb = cst.tile([128, 2], F32, tag="bk")
            nc.sync.dma_start(out=bk_sb, in_=bkp[:, :])
            # bv broadcast to all partitions: [128, 256]
            bv_sb = cst.tile([128, OC], F32, tag="bv")
            nc.gpsimd.dma_start(out=bv_sb, in_=bvf[:].partition_broadcast(128))

            # ---- big resident inputs ----
            hT_sb = []
            for e in range(8):
                t = cst.tile([128, S], BF16, tag=f"hT{e}")
                nc.sync.dma_start(out=t, in_=hT[128 * e:128 * (e + 1), :])
                hT_sb.append(t)
            w_sb = {}
            for nm, src in (("q", wqT), ("k", wkT), ("v", wvT)):
                lst = []
                for e in range(8):
                    t = cst.tile([128, OC], BF16, tag=f"w{nm}{e}")
                    nc.sync.dma_start(out=t, in_=src[128 * e:128 * (e + 1), :])
                    lst.append(t)
                w_sb[nm] = lst

            # ---- persistent intermediates ----
            qT = [cst.tile([128, S], BF16, tag=f"qT{p}", name=f"qT{p}") for p in range(NPAIR)]
            kT = [cst.tile([128, S], BF16, tag=f"kT{p}", name=f"kT{p}") for p in range(NPAIR)]
            vaug = [[cst.tile([128, 130], BF16, tag=f"va{p}_{tt}", name=f"va{p}_{tt}")
                     for tt in range(NT)] for p in range(NPAIR)]
            outsb = [cst.tile([128, OC], F32, tag=f"o{tt}", name=f"o{tt}") for tt in range(NT)]

            def emit_qk_proj(p):
                po = 128 * p
                # q blocks descending (attention consumes j=3 first),
                # k blocks ascending (attention consumes tk tiles in order)
                for nm, dst, b_sb, scl, order in (
                        ("q", qT[p], bq_sb, 0.125, reversed(range(NT4))),
                        ("k", kT[p], bk_sb, None, range(NT4))):
                    for t4 in order:
                        ts = slice(512 * t4, 512 * (t4 + 1))
                        ps_qk = ps_small.tile([128, 512], F32, tag="sm", name="ps_qk")
                        for e in range(8):
                            nc.tensor.matmul(
                                ps_qk,
                                w_sb[nm][e][:, po:po + 128],
                                hT_sb[e][:, ts],
                                start=(e == 0), stop=(e == 7),
                            )
                        if scl is not None:
                            nc.vector.tensor_scalar(
                                out=dst[:, ts], in0=ps_qk,
                                scalar1=scl, scalar2=b_sb[:, p:p + 1],
                                op0=MULT, op1=ADD,
                            )
                        else:
                            nc.vector.tensor_scalar_add(
                                out=dst[:, ts], in0=ps_qk, scalar1=b_sb[:, p:p + 1],
                            )

            def emit_v_proj():
                for tt in range(NT):
                    rs = slice(128 * tt, 128 * (tt + 1))
                    ps_v = ps_small.tile([128, OC], F32, tag="sm", name="ps_v")
                    for e in range(8):
                        nc.tensor.matmul(
                            ps_v,
                            hT_sb[e][:, rs],
                            w_sb["v"][e][:, :],
                            start=(e == 0), stop=(e == 7),
                        )
                    for p in range(NPAIR):
                        po = 128 * p
                        vt = vaug[p][tt]
                        vt3 = vt.rearrange("a (h c) -> a h c", h=2)[:, :, 0:64]
                        ps3 = ps_v[:, po:po + 128].rearrange("a (h c) -> a h c", h=2)
                        bv3 = bv_sb[:, po:po + 128].rearrange("a (h c) -> a h c", h=2)
                        nc.vector.tensor_add(vt3, ps3, bv3)
                        nc.vector.memset(
                            vt.rearrange("a (h c) -> a h c", h=2)[:, :, 64:65], 1.0)

            def emit_attention(p):
                po = 128 * p
                for j in reversed(range(NT4)):
                    qs = slice(512 * j, 512 * (j + 1))
                    out_A = ps_out.tile([65, 512], F32, tag="out", name="out_A")
                    out_B = ps_out.tile([65, 512], F32, tag="out", name="out_B")
                    ntk = 4 * (j + 1)
                    for i in range(ntk):
                        ks = slice(128 * i, 128 * (i + 1))
                        sc = ps_sc.tile([128, 1024], F32, tag="sc", name="sc")
                        nc.tensor.matmul(sc[:, 0:512], kT[p][0:64, ks],
                                         qT[p][0:64, qs], start=True, stop=True)
                        nc.tensor.matmul(sc[:, 512:1024], kT[p][64:128, ks],
                                         qT[p][64:128, qs], start=True, stop=True)
                        ex = expp.tile([128, 1024], BF16, tag="exp", name="ex")
                        nc.scalar.activation(out=ex, in_=sc, func=EXP,
                                             bias=mask_sb[:, i:i + 1], scale=1.0)
                        if i >= 4 * j:
                            s_off = 128 * i - 512 * j
                            ex3 = ex.rearrange("a (h f) -> a h f", h=2)
                            nc.gpsimd.affine_select(
                                out=ex3, in_=ex3, compare_op=GE, fill=0.0,
                                base=-s_off, pattern=[[0, 2], [1, 512]],
                                channel_multiplier=-1,
                            )
                        nc.tensor.matmul(out_A, vaug[p][i][:, 0:65],
                                         ex[:, 0:512],
                                         start=(i == 0), stop=(i == ntk - 1))
                        nc.tensor.matmul(out_B, vaug[p][i][:, 65:130],
                                         ex[:, 512:1024],
                                         start=(i == 0), stop=(i == ntk - 1))

                    for h_loc, out_ps in ((0, out_A), (1, out_B)):
                        u = work.tile([65, 512], F32, tag="u", name="u")
                        nc.vector.tensor_copy(u, out_ps)
                        for s4 in range(4):
                            tp = ps_small.tile([128, 65], F32, tag="sm", name="tp")
                            nc.tensor.transpose(tp, u[:, 128 * s4:128 * (s4 + 1)],
                                                ident_sb)
                            r = work.tile([128, 1], F32, tag="r", name="r")
                            nc.vector.reciprocal(r, tp[:, 64:65])
                            tt = 4 * j + s4
                            c0 = po + 64 * h_loc
                            nc.vector.tensor_scalar_mul(
                                outsb[tt][:, c0:c0 + 64], tp[:, 0:64], r)
                    if p == NPAIR - 1:
                        for s4 in range(4):
                            tt = 4 * j + s4
                            nc.sync.dma_start(
                                out=out[128 * tt:128 * (tt + 1), :],
                                in_=outsb[tt])

            emit_qk_proj(0)
            emit_v_proj()
            emit_attention(0)
            emit_qk_proj(1)
            emit_attention(1)

    nc.compile()
    return nc


def _get_nc():
    global _cached_nc
    if _cached_nc is None:
        _cached_nc = _build()
    return _cached_nc


def kernel(hidden_states, attention_mask, Wq, bq, Wk, bk, Wv, bv):
    hidden_states = np.asarray(hidden_states, dtype=np.float32)
    attention_mask = np.asarray(attention_mask, dtype=np.float32)
    Wq = np.asarray(Wq, dtype=np.float32)
    Wk = np.asarray(Wk, dtype=np.float32)
    Wv = np.asarray(Wv, dtype=np.float32)
    bq = np.asarray(bq, dtype=np.float32)
    bk = np.asarray(bk, dtype=np.float32)
    bv = np.asarray(bv, dtype=np.float32)

    bf = ml_dtypes.bfloat16
    ident = np.eye(65, dtype=np.float32)
    in_maps = []
    for c in range(NCORES):
        b, g = divmod(c, 4)
        cs = slice(OC * g, OC * (g + 1))
        in_maps.append({
            "hT": np.ascontiguousarray(hidden_states[b].T).astype(bf),
            "wqT": np.ascontiguousarray(Wq[cs, :].T).astype(bf),
            "wkT": np.ascontiguousarray(Wk[cs, :].T).astype(bf),
            "wvT": np.ascontiguousarray(Wv[cs, :].T).astype(bf),
            "bqp": np.ascontiguousarray(bq[cs].reshape(2, 128).T),
            "bkp": np.ascontiguousarray(bk[cs].reshape(2, 128).T),
            "bvf": np.ascontiguousarray(bv[cs]),
            "mask_t": np.ascontiguousarray(
                attention_mask[b, 0, 0, :].reshape(NT, 128).T),
            "ident": ident,
        })

    nc = _get_nc()
    res = run_bass_kernel_spmd(nc, in_maps, list(range(NCORES)))

    full = np.empty((B, S, H * D), dtype=np.float32)
    for c in range(NCORES):
        b, g = divmod(c, 4)
        full[b, :, OC * g:OC * (g + 1)] = res.results[c]["out"]
    return full


# revision 6
# speedup vs baseline: 1.0351x; 1.0351x over previous
"""Causal self-attention (B=2, S=2048, E=1024, H=16, D=64) on 8 trn2 NeuronCores.

Sharding: core c = (batch b = c // 4, head-group g = c % 4).  Each core computes
4 heads (one quarter of the 16) for one batch: projections q/k/v for its 256
output channels, then causal flash-style attention, writing out[b, :, 256g:256g+256].

Per-core kernel design (Bass/Tile):
  - Host pre-transposes hidden -> hT [E, S] (bf16) and weight slices -> wT [E, 256]
    (bf16) so all matmul contractions have K on partitions.
  - q/k projections (bf16, PSUM-accumulated over 8 E-chunks) produce qT/kT in
    [d, t] layout, copied to SBUF as float32r with scale 1/8 (q) and bias add.
  - v projection produces v in [t, d] layout; DVE copy splits heads into
    v_aug tiles [tk=128, 65*2] with a ones column per head (sum-of-exp trick).
  - scores^T tiles [tk=128, tq=512] per head via single f32r matmuls; the two
    heads of a pair run concurrently on PE row halves (K=64 each).
  - exp via ScalarE activation (attention-mask bias per tk partition), bf16 out.
  - causal masking: gpsimd affine_select zeroes the invalid region of
    diagonal-crossing tiles after exp.
  - attn @ v_aug accumulates unnormalized out^T [65, tq] in PSUM (bf16 matmuls);
    row 64 is the softmax denominator.
  - PE transpose [65,128] -> [128,65], then DVE reciprocal + tensor_scalar mul
    normalizes and writes [t, d] output tiles; DMA to DRAM.
"""

import numpy as np
import ml_dtypes

import concourse.bass as bass
import concourse.mybir as mybir
import concourse.tile as tile
from concourse import bacc
from concourse.bass_utils import run_bass_kernel_spmd

F32 = mybir.dt.float32
F32R = mybir.dt.float32r
BF16 = mybir.dt.bfloat16

B, S, E = 2, 2048, 1024
H, D = 16, 64
NCORES = 8
OC = 256          # output channels per core (4 heads)
NPAIR = 2         # head pairs per core
NT = S // 128     # 16 tk tiles
NT4 = S // 512    # 4 tq blocks

_cached_nc = None


def _build():
    nc = bacc.Bacc()

    hT = nc.declare_dram_parameter("hT", [E, S], BF16, isOutput=False)
    wqT = nc.declare_dram_parameter("wqT", [E, OC], BF16, isOutput=False)
    wkT = nc.declare_dram_parameter("wkT", [E, OC], BF16, isOutput=False)
    wvT = nc.declare_dram_parameter("wvT", [E, OC], BF16, isOutput=False)
    bqp = nc.declare_dram_parameter("bqp", [128, 2], F32, isOutput=False)
    bkp = nc.declare_dram_parameter("bkp", [128, 2], F32, isOutput=False)
    bvf = nc.declare_dram_parameter("bvf", [OC], F32, isOutput=False)
    mask_t = nc.declare_dram_parameter("mask_t", [128, NT], F32, isOutput=False)
    ident = nc.declare_dram_parameter("ident", [65, 65], F32, isOutput=False)
    out = nc.declare_dram_parameter("out", [S, OC], F32, isOutput=True)

    EXP = mybir.ActivationFunctionType.Exp
    ADD = mybir.AluOpType.add
    MULT = mybir.AluOpType.mult
    GE = mybir.AluOpType.is_ge

    with tile.TileContext(nc) as tc:
        with (
            tc.tile_pool(name="cst", bufs=1) as cst,
            tc.tile_pool(name="work", bufs=3) as work,
            tc.tile_pool(name="expp", bufs=6) as expp,
            tc.tile_pool(name="ps_small", bufs=2, space="PSUM") as ps_small,
            tc.tile_pool(name="ps_sc", bufs=2, space="PSUM") as ps_sc,
            tc.tile_pool(name="ps_out", bufs=2, space="PSUM") as ps_out,
        ):
            # ---- constants ----
            mask_sb = cst.tile([128, NT], F32, tag="mask")
            nc.sync.dma_start(out=mask_sb, in_=mask_t[:, :])
            ident_sb = cst.tile([65, 65], F32, tag="ident")
            nc.sync.dma_start(out=ident_sb, in_=ident[:, :])
            bq_sb = cst.tile([128, 2], F32, tag="bq")
            nc.sync.dma_start(out=bq_sb, in_=bqp[:, :])
            bk_sb = cst.tile([128, 2], F32, tag="bk")
            nc.sync.dma_start(out=bk_sb, in_=bkp[:, :])
            # bv broadcast to all partitions: [128, 256]
            bv_sb = cst.tile([128, OC], F32, tag="bv")
            nc.gpsimd.dma_start(out=bv_sb, in_=bvf[:].partition_broadcast(128))

            # ---- big resident inputs (wq/hT interleaved so the first
            # q-projection chain can start while later chunks stream in) ----
            hT_sb = []
            w_sb = {"q": [], "k": [], "v": []}
            for e in range(8):
                wt = cst.tile([128, OC], BF16, tag=f"wq{e}", name=f"wq{e}")
                nc.sync.dma_start(out=wt, in_=wqT[128 * e:128 * (e + 1), :])
                w_sb["q"].append(wt)
                t = cst.tile([128, S], BF16, tag=f"hT{e}", name=f"hT{e}")
                nc.sync.dma_start(out=t, in_=hT[128 * e:128 * (e + 1), :])
                hT_sb.append(t)
            for nm, srcp in (("k", wkT), ("v", wvT)):
                for e in range(8):
                    wt = cst.tile([128, OC], BF16, tag=f"w{nm}{e}", name=f"w{nm}{e}")
                    nc.sync.dma_start(out=wt, in_=srcp[128 * e:128 * (e + 1), :])
                    w_sb[nm].append(wt)

            # ---- persistent intermediates ----
            qT = [cst.tile([128, S], BF16, tag=f"qT{p}", name=f"qT{p}") for p in range(NPAIR)]
            kT = [cst.tile([128, S], BF16, tag=f"kT{p}", name=f"kT{p}") for p in range(NPAIR)]
            vaug = [[cst.tile([128, 130], BF16, tag=f"va{p}_{tt}", name=f"va{p}_{tt}")
                     for tt in range(NT)] for p in range(NPAIR)]
            outsb = [cst.tile([128, OC], F32, tag=f"o{tt}", name=f"o{tt}") for tt in range(NT)]

            def emit_qk_chain(nm, p, t4):
                po = 128 * p
                dst = qT[p] if nm == "q" else kT[p]
                b_sb = bq_sb if nm == "q" else bk_sb
                ts = slice(512 * t4, 512 * (t4 + 1))
                ps_qk = ps_small.tile([128, 512], F32, tag="sm", name="ps_qk")
                for e in range(8):
                    nc.tensor.matmul(
                        ps_qk,
                        w_sb[nm][e][:, po:po + 128],
                        hT_sb[e][:, ts],
                        start=(e == 0), stop=(e == 7),
                    )
                if nm == "q":
                    nc.vector.tensor_scalar(
                        out=dst[:, ts], in0=ps_qk,
                        scalar1=0.125, scalar2=b_sb[:, p:p + 1],
                        op0=MULT, op1=ADD,
                    )
                else:
                    nc.vector.tensor_scalar_add(
                        out=dst[:, ts], in0=ps_qk, scalar1=b_sb[:, p:p + 1],
                    )

            def emit_v_chain(tt):
                rs = slice(128 * tt, 128 * (tt + 1))
                ps_v = ps_small.tile([128, OC], F32, tag="sm", name="ps_v")
                for e in range(8):
                    nc.tensor.matmul(
                        ps_v,
                        hT_sb[e][:, rs],
                        w_sb["v"][e][:, :],
                        start=(e == 0), stop=(e == 7),
                    )
                for p in range(NPAIR):
                    po = 128 * p
                    vt = vaug[p][tt]
                    vt3 = vt.rearrange("a (h c) -> a h c", h=2)[:, :, 0:64]
                    ps3 = ps_v[:, po:po + 128].rearrange("a (h c) -> a h c", h=2)
                    bv3 = bv_sb[:, po:po + 128].rearrange("a (h c) -> a h c", h=2)
                    nc.vector.tensor_add(vt3, ps3, bv3)
                    nc.vector.memset(
                        vt.rearrange("a (h c) -> a h c", h=2)[:, :, 64:65], 1.0)

            def emit_attention(p, chores=()):
                chores = list(chores)
                po = 128 * p
                for j in reversed(range(NT4)):
                    qs = slice(512 * j, 512 * (j + 1))
                    out_A = ps_out.tile([65, 512], F32, tag="out", name="out_A")
                    out_B = ps_out.tile([65, 512], F32, tag="out", name="out_B")
                    ntk = 4 * (j + 1)
                    for i in range(ntk):
                        ks = slice(128 * i, 128 * (i + 1))
                        sc = ps_sc.tile([128, 1024], F32, tag="sc", name="sc")
                        nc.tensor.matmul(sc[:, 0:512], kT[p][0:64, ks],
                                         qT[p][0:64, qs], start=True, stop=True)
                        nc.tensor.matmul(sc[:, 512:1024], kT[p][64:128, ks],
                                         qT[p][64:128, qs], start=True, stop=True)
                        ex = expp.tile([128, 1024], BF16, tag="exp", name="ex")
                        nc.scalar.activation(out=ex, in_=sc, func=EXP,
                                             bias=mask_sb[:, i:i + 1], scale=1.0)
                        if i >= 4 * j:
                            s_off = 128 * i - 512 * j
                            ex3 = ex.rearrange("a (h f) -> a h f", h=2)
                            nc.gpsimd.affine_select(
                                out=ex3, in_=ex3, compare_op=GE, fill=0.0,
                                base=-s_off, pattern=[[0, 2], [1, 512]],
                                channel_multiplier=-1,
                            )
                        nc.tensor.matmul(out_A, vaug[p][i][:, 0:65],
                                         ex[:, 0:512],
                                         start=(i == 0), stop=(i == ntk - 1))
                        nc.tensor.matmul(out_B, vaug[p][i][:, 65:130],
                                         ex[:, 512:1024],
                                         start=(i == 0), stop=(i == ntk - 1))
                        if chores:
                            chores.pop(0)()

                    for h_loc, out_ps in ((0, out_A), (1, out_B)):
                        u = work.tile([65, 512], F32, tag="u", name="u")
                        nc.vector.tensor_copy(u, out_ps)
                        for s4 in range(4):
                            tp = ps_small.tile([128, 65], F32, tag="sm", name="tp")
                            nc.tensor.transpose(tp, u[:, 128 * s4:128 * (s4 + 1)],
                                                ident_sb)
                            r = work.tile([128, 1], F32, tag="r", name="r")
                            nc.vector.reciprocal(r, tp[:, 64:65])
                            tt = 4 * j + s4
                            c0 = po + 64 * h_loc
                            nc.vector.tensor_scalar_mul(
                                outsb[tt][:, c0:c0 + 64], tp[:, 0:64], r)
                    if p == NPAIR - 1:
                        for s4 in range(4):
                            tt = 4 * j + s4
                            nc.sync.dma_start(
                                out=out[128 * tt:128 * (tt + 1), :],
                                in_=outsb[tt])

            # prologue: minimal producers for attention(p0, j=3, i=0..)
            emit_qk_chain("q", 0, 3)
            emit_qk_chain("k", 0, 0)
            for tt in range(4):
                emit_v_chain(tt)
            chores = []
            chores.append(lambda: emit_qk_chain("k", 0, 1))
            chores.append(lambda: emit_v_chain(4))
            chores.append(lambda: emit_v_chain(5))
            chores.append(lambda: emit_v_chain(6))
            chores.append(lambda: emit_qk_chain("k", 0, 2))
            chores.append(lambda: emit_v_chain(7))
            chores.append(lambda: emit_v_chain(8))
            chores.append(lambda: emit_v_chain(9))
            chores.append(lambda: emit_qk_chain("k", 0, 3))
            for _tt in range(10, 16):
                chores.append(lambda _tt=_tt: emit_v_chain(_tt))
            chores.append(lambda: emit_qk_chain("q", 0, 2))
            chores.append(lambda: emit_qk_chain("q", 0, 1))
            chores.append(lambda: emit_qk_chain("q", 0, 0))
            for t4 in (3, 2, 1, 0):
                chores.append(lambda t4=t4: emit_qk_chain("q", 1, t4))
                chores.append(lambda t4=t4: emit_qk_chain("k", 1, 3 - t4))
            emit_attention(0, chores)
            emit_attention(1)

    nc.compile()
    return nc


def _get_nc():
    global _cached_nc
    if _cached_nc is None:
        _cached_nc = _build()
    return _cached_nc


def kernel(hidden_states, attention_mask, Wq, bq, Wk, bk, Wv, bv):
    hidden_states = np.asarray(hidden_states, dtype=np.float32)
    attention_mask = np.asarray(attention_mask, dtype=np.float32)
    Wq = np.asarray(Wq, dtype=np.float32)
    Wk = np.asarray(Wk, dtype=np.float32)
    Wv = np.asarray(Wv, dtype=np.float32)
    bq = np.asarray(bq, dtype=np.float32)
    bk = np.asarray(bk, dtype=np.float32)
    bv = np.asarray(bv, dtype=np.float32)

    bf = ml_dtypes.bfloat16
    ident = np.eye(65, dtype=np.float32)
    in_maps = []
    for c in range(NCORES):
        b, g = divmod(c, 4)
        cs = slice(OC * g, OC * (g + 1))
        in_maps.append({
            "hT": np.ascontiguousarray(hidden_states[b].T).astype(bf),
            "wqT": np.ascontiguousarray(Wq[cs, :].T).astype(bf),
            "wkT": np.ascontiguousarray(Wk[cs, :].T).astype(bf),
            "wvT": np.ascontiguousarray(Wv[cs, :].T).astype(bf),
            "bqp": np.ascontiguousarray(bq[cs].reshape(2, 128).T),
            "bkp": np.ascontiguousarray(bk[cs].reshape(2, 128).T),
            "bvf": np.ascontiguousarray(bv[cs]),
            "mask_t": np.ascontiguousarray(
                attention_mask[b, 0, 0, :].reshape(NT, 128).T),
            "ident": ident,
        })

    nc = _get_nc()
    res = run_bass_kernel_spmd(nc, in_maps, list(range(NCORES)))

    full = np.empty((B, S, H * D), dtype=np.float32)
    for c in range(NCORES):
        b, g = divmod(c, 4)
        full[b, :, OC * g:OC * (g + 1)] = res.results[c]["out"]
    return full


# revision 7
# speedup vs baseline: 1.0356x; 1.0004x over previous
"""Causal self-attention (B=2, S=2048, E=1024, H=16, D=64) on 8 trn2 NeuronCores.

Sharding: core c = (batch b = c // 4, head-group g = c % 4).  Each core computes
4 heads (one quarter of the 16) for one batch: projections q/k/v for its 256
output channels, then causal flash-style attention, writing out[b, :, 256g:256g+256].

Per-core kernel design (Bass/Tile):
  - Host pre-transposes hidden -> hT [E, S] (bf16) and weight slices -> wT [E, 256]
    (bf16) so all matmul contractions have K on partitions.
  - q/k projections (bf16, PSUM-accumulated over 8 E-chunks) produce qT/kT in
    [d, t] layout, copied to SBUF as float32r with scale 1/8 (q) and bias add.
  - v projection produces v in [t, d] layout; DVE copy splits heads into
    v_aug tiles [tk=128, 65*2] with a ones column per head (sum-of-exp trick).
  - scores^T tiles [tk=128, tq=512] per head via single f32r matmuls; the two
    heads of a pair run concurrently on PE row halves (K=64 each).
  - exp via ScalarE activation (attention-mask bias per tk partition), bf16 out.
  - causal masking: gpsimd affine_select zeroes the invalid region of
    diagonal-crossing tiles after exp.
  - attn @ v_aug accumulates unnormalized out^T [65, tq] in PSUM (bf16 matmuls);
    row 64 is the softmax denominator.
  - PE transpose [65,128] -> [128,65], then DVE reciprocal + tensor_scalar mul
    normalizes and writes [t, d] output tiles; DMA to DRAM.
"""

import numpy as np
import ml_dtypes

import concourse.bass as bass
import concourse.mybir as mybir
import concourse.tile as tile
from concourse import bacc
from concourse.bass_utils import run_bass_kernel_spmd

F32 = mybir.dt.float32
F32R = mybir.dt.float32r
BF16 = mybir.dt.bfloat16

B, S, E = 2, 2048, 1024
H, D = 16, 64
NCORES = 8
OC = 256          # output channels per core (4 heads)
NPAIR = 2         # head pairs per core
NT = S // 128     # 16 tk tiles
NT4 = S // 512    # 4 tq blocks

_cached_nc = None


def _build():
    nc = bacc.Bacc()

    hT = nc.declare_dram_parameter("hT", [E, S], BF16, isOutput=False)
    wqT = nc.declare_dram_parameter("wqT", [E, OC], BF16, isOutput=False)
    wkT = nc.declare_dram_parameter("wkT", [E, OC], BF16, isOutput=False)
    wvT = nc.declare_dram_parameter("wvT", [E, OC], BF16, isOutput=False)
    bqp = nc.declare_dram_parameter("bqp", [128, 2], F32, isOutput=False)
    bkp = nc.declare_dram_parameter("bkp", [128, 2], F32, isOutput=False)
    bvf = nc.declare_dram_parameter("bvf", [OC], F32, isOutput=False)
    mask_t = nc.declare_dram_parameter("mask_t", [128, NT], F32, isOutput=False)
    ident = nc.declare_dram_parameter("ident", [65, 65], BF16, isOutput=False)
    out = nc.declare_dram_parameter("out", [S, OC], F32, isOutput=True)

    EXP = mybir.ActivationFunctionType.Exp
    ADD = mybir.AluOpType.add
    MULT = mybir.AluOpType.mult
    GE = mybir.AluOpType.is_ge

    with tile.TileContext(nc) as tc:
        with (
            tc.tile_pool(name="cst", bufs=1) as cst,
            tc.tile_pool(name="work", bufs=3) as work,
            tc.tile_pool(name="expp", bufs=6) as expp,
            tc.tile_pool(name="ps_small", bufs=2, space="PSUM") as ps_small,
            tc.tile_pool(name="ps_sc", bufs=2, space="PSUM") as ps_sc,
            tc.tile_pool(name="ps_out", bufs=2, space="PSUM") as ps_out,
        ):
            # ---- constants ----
            mask_sb = cst.tile([128, NT], F32, tag="mask")
            nc.sync.dma_start(out=mask_sb, in_=mask_t[:, :])
            ident_sb = cst.tile([65, 65], BF16, tag="ident")
            nc.sync.dma_start(out=ident_sb, in_=ident[:, :])
            bq_sb = cst.tile([128, 2], F32, tag="bq")
            nc.sync.dma_start(out=bq_sb, in_=bqp[:, :])
            bk_sb = cst.tile([128, 2], F32, tag="bk")
            nc.sync.dma_start(out=bk_sb, in_=bkp[:, :])
            # bv broadcast to all partitions: [128, 256]
            bv_sb = cst.tile([128, OC], F32, tag="bv")
            nc.gpsimd.dma_start(out=bv_sb, in_=bvf[:].partition_broadcast(128))

            # ---- big resident inputs; hT split [e][t4] and DMA'd in the
            # order the prologue consumes it (q block3, k block0, rest) ----
            hT32 = [[None] * NT4 for _ in range(8)]
            w_sb = {"q": [], "k": [], "v": []}

            def dma_ht(e, t4):
                t = cst.tile([128, 512], BF16, tag=f"hT{e}_{t4}", name=f"hT{e}_{t4}")
                nc.sync.dma_start(
                    out=t, in_=hT[128 * e:128 * (e + 1), 512 * t4:512 * (t4 + 1)])
                hT32[e][t4] = t

            for e in range(8):
                wt = cst.tile([128, OC], BF16, tag=f"wq{e}", name=f"wq{e}")
                nc.sync.dma_start(out=wt, in_=wqT[128 * e:128 * (e + 1), :])
                w_sb["q"].append(wt)
                dma_ht(e, 3)
            for e in range(8):
                wt = cst.tile([128, OC], BF16, tag=f"wk{e}", name=f"wk{e}")
                nc.sync.dma_start(out=wt, in_=wkT[128 * e:128 * (e + 1), :])
                w_sb["k"].append(wt)
                dma_ht(e, 0)
            for e in range(8):
                wt = cst.tile([128, OC], BF16, tag=f"wv{e}", name=f"wv{e}")
                nc.sync.dma_start(out=wt, in_=wvT[128 * e:128 * (e + 1), :])
                w_sb["v"].append(wt)
            for t4 in (1, 2):
                for e in range(8):
                    dma_ht(e, t4)

            # ---- persistent intermediates ----
            qT = [cst.tile([128, S], BF16, tag=f"qT{p}", name=f"qT{p}") for p in range(NPAIR)]
            kT = [cst.tile([128, S], BF16, tag=f"kT{p}", name=f"kT{p}") for p in range(NPAIR)]
            vaug = [[cst.tile([128, 130], BF16, tag=f"va{p}_{tt}", name=f"va{p}_{tt}")
                     for tt in range(NT)] for p in range(NPAIR)]
            outsb = [cst.tile([128, OC], F32, tag=f"o{tt}", name=f"o{tt}") for tt in range(NT)]

            def emit_qk_chain(nm, p, t4):
                po = 128 * p
                dst = qT[p] if nm == "q" else kT[p]
                b_sb = bq_sb if nm == "q" else bk_sb
                ts = slice(512 * t4, 512 * (t4 + 1))
                ps_qk = ps_small.tile([128, 512], F32, tag="sm", name="ps_qk")
                for e in range(8):
                    nc.tensor.matmul(
                        ps_qk,
                        w_sb[nm][e][:, po:po + 128],
                        hT32[e][t4],
                        start=(e == 0), stop=(e == 7),
                    )
                if nm == "q":
                    nc.vector.tensor_scalar(
                        out=dst[:, ts], in0=ps_qk,
                        scalar1=0.125, scalar2=b_sb[:, p:p + 1],
                        op0=MULT, op1=ADD,
                    )
                else:
                    nc.vector.tensor_scalar_add(
                        out=dst[:, ts], in0=ps_qk, scalar1=b_sb[:, p:p + 1],
                    )

            def emit_v_chain(tt):
                t4v, r4 = divmod(tt, 4)
                rs = slice(128 * r4, 128 * (r4 + 1))
                ps_v = ps_small.tile([128, OC], F32, tag="sm", name="ps_v")
                for e in range(8):
                    nc.tensor.matmul(
                        ps_v,
                        hT32[e][t4v][:, rs],
                        w_sb["v"][e][:, :],
                        start=(e == 0), stop=(e == 7),
                    )
                for p in range(NPAIR):
                    po = 128 * p
                    vt = vaug[p][tt]
                    vt3 = vt.rearrange("a (h c) -> a h c", h=2)[:, :, 0:64]
                    ps3 = ps_v[:, po:po + 128].rearrange("a (h c) -> a h c", h=2)
                    bv3 = bv_sb[:, po:po + 128].rearrange("a (h c) -> a h c", h=2)
                    nc.vector.tensor_add(vt3, ps3, bv3)
                    nc.vector.memset(
                        vt.rearrange("a (h c) -> a h c", h=2)[:, :, 64:65], 1.0)

            def emit_attention(p, chores=()):
                chores = list(chores)
                po = 128 * p
                for j in reversed(range(NT4)):
                    qs = slice(512 * j, 512 * (j + 1))
                    out_A = ps_out.tile([65, 512], F32, tag="out", name="out_A")
                    out_B = ps_out.tile([65, 512], F32, tag="out", name="out_B")
                    ntk = 4 * (j + 1)
                    for i in range(ntk):
                        ks = slice(128 * i, 128 * (i + 1))
                        sc = ps_sc.tile([128, 1024], F32, tag="sc", name="sc")
                        nc.tensor.matmul(sc[:, 0:512], kT[p][0:64, ks],
                                         qT[p][0:64, qs], start=True, stop=True)
                        nc.tensor.matmul(sc[:, 512:1024], kT[p][64:128, ks],
                                         qT[p][64:128, qs], start=True, stop=True)
                        ex = expp.tile([128, 1024], BF16, tag="exp", name="ex")
                        nc.scalar.activation(out=ex, in_=sc, func=EXP,
                                             bias=mask_sb[:, i:i + 1], scale=1.0)
                        if i >= 4 * j:
                            s_off = 128 * i - 512 * j
                            ex3 = ex.rearrange("a (h f) -> a h f", h=2)
                            nc.gpsimd.affine_select(
                                out=ex3, in_=ex3, compare_op=GE, fill=0.0,
                                base=-s_off, pattern=[[0, 2], [1, 512]],
                                channel_multiplier=-1,
                            )
                        nc.tensor.matmul(out_A, vaug[p][i][:, 0:65],
                                         ex[:, 0:512],
                                         start=(i == 0), stop=(i == ntk - 1))
                        nc.tensor.matmul(out_B, vaug[p][i][:, 65:130],
                                         ex[:, 512:1024],
                                         start=(i == 0), stop=(i == ntk - 1))
                        if chores:
                            chores.pop(0)()

                    for h_loc, out_ps in ((0, out_A), (1, out_B)):
                        u = work.tile([65, 512], BF16, tag="u", name="u")
                        nc.vector.tensor_copy(u, out_ps)
                        for s4 in range(4):
                            tp = ps_small.tile([128, 65], BF16, tag="sm", name="tp")
                            nc.tensor.transpose(tp, u[:, 128 * s4:128 * (s4 + 1)],
                                                ident_sb)
                            r = work.tile([128, 1], F32, tag="r", name="r")
                            nc.vector.reciprocal(r, tp[:, 64:65])
                            tt = 4 * j + s4
                            c0 = po + 64 * h_loc
                            nc.vector.tensor_scalar_mul(
                                outsb[tt][:, c0:c0 + 64], tp[:, 0:64], r)
                    if p == NPAIR - 1:
                        for s4 in range(4):
                            tt = 4 * j + s4
                            nc.sync.dma_start(
                                out=out[128 * tt:128 * (tt + 1), :],
                                in_=outsb[tt])

            # prologue: minimal producers for attention(p0, j=3, i=0..)
            emit_qk_chain("q", 0, 3)
            emit_qk_chain("k", 0, 0)
            for tt in range(4):
                emit_v_chain(tt)
            chores = []
            chores.append(lambda: emit_qk_chain("k", 0, 1))
            chores.append(lambda: emit_v_chain(4))
            chores.append(lambda: emit_v_chain(5))
            chores.append(lambda: emit_v_chain(6))
            chores.append(lambda: emit_qk_chain("k", 0, 2))
            chores.append(lambda: emit_v_chain(7))
            chores.append(lambda: emit_v_chain(8))
            chores.append(lambda: emit_v_chain(9))
            chores.append(lambda: emit_qk_chain("k", 0, 3))
            for _tt in range(10, 16):
                chores.append(lambda _tt=_tt: emit_v_chain(_tt))
            chores.append(lambda: emit_qk_chain("q", 0, 2))
            chores.append(lambda: emit_qk_chain("q", 0, 1))
            chores.append(lambda: emit_qk_chain("q", 0, 0))
            for t4 in (3, 2, 1, 0):
                chores.append(lambda t4=t4: emit_qk_chain("q", 1, t4))
                chores.append(lambda t4=t4: emit_qk_chain("k", 1, 3 - t4))
            emit_attention(0, chores)
            emit_attention(1)

    nc.compile()
    return nc


def _get_nc():
    global _cached_nc
    if _cached_nc is None:
        _cached_nc = _build()
    return _cached_nc


def kernel(hidden_states, attention_mask, Wq, bq, Wk, bk, Wv, bv):
    hidden_states = np.asarray(hidden_states, dtype=np.float32)
    attention_mask = np.asarray(attention_mask, dtype=np.float32)
    Wq = np.asarray(Wq, dtype=np.float32)
    Wk = np.asarray(Wk, dtype=np.float32)
    Wv = np.asarray(Wv, dtype=np.float32)
    bq = np.asarray(bq, dtype=np.float32)
    bk = np.asarray(bk, dtype=np.float32)
    bv = np.asarray(bv, dtype=np.float32)

    bf = ml_dtypes.bfloat16
    ident = np.eye(65, dtype=np.float32).astype(ml_dtypes.bfloat16)
    in_maps = []
    for c in range(NCORES):
        b, g = divmod(c, 4)
        cs = slice(OC * g, OC * (g + 1))
        in_maps.append({
            "hT": np.ascontiguousarray(hidden_states[b].T).astype(bf),
            "wqT": np.ascontiguousarray(Wq[cs, :].T).astype(bf),
            "wkT": np.ascontiguousarray(Wk[cs, :].T).astype(bf),
            "wvT": np.ascontiguousarray(Wv[cs, :].T).astype(bf),
            "bqp": np.ascontiguousarray(bq[cs].reshape(2, 128).T),
            "bkp": np.ascontiguousarray(bk[cs].reshape(2, 128).T),
            "bvf": np.ascontiguousarray(bv[cs]),
            "mask_t": np.ascontiguousarray(
                attention_mask[b, 0, 0, :].reshape(NT, 128).T),
            "ident": ident,
        })

    nc = _get_nc()
    res = run_bass_kernel_spmd(nc, in_maps, list(range(NCORES)))

    full = np.empty((B, S, H * D), dtype=np.float32)
    for c in range(NCORES):
        b, g = divmod(c, 4)
        full[b, :, OC * g:OC * (g + 1)] = res.results[c]["out"]
    return full


# revision 8
# speedup vs baseline: 1.0373x; 1.0016x over previous
"""Causal self-attention (B=2, S=2048, E=1024, H=16, D=64) on 8 trn2 NeuronCores.

Sharding: core c = (batch b = c // 4, head-group g = c % 4).  Each core computes
4 heads (one quarter of the 16) for one batch: projections q/k/v for its 256
output channels, then causal flash-style attention, writing out[b, :, 256g:256g+256].

Per-core kernel design (Bass/Tile):
  - Host pre-transposes hidden -> hT [E, S] (bf16) and weight slices -> wT [E, 256]
    (bf16) so all matmul contractions have K on partitions.
  - q/k projections (bf16, PSUM-accumulated over 8 E-chunks) produce qT/kT in
    [d, t] layout, copied to SBUF as float32r with scale 1/8 (q) and bias add.
  - v projection produces v in [t, d] layout; DVE copy splits heads into
    v_aug tiles [tk=128, 65*2] with a ones column per head (sum-of-exp trick).
  - scores^T tiles [tk=128, tq=512] per head via single f32r matmuls; the two
    heads of a pair run concurrently on PE row halves (K=64 each).
  - exp via ScalarE activation (attention-mask bias per tk partition), bf16 out.
  - causal masking: gpsimd affine_select zeroes the invalid region of
    diagonal-crossing tiles after exp.
  - attn @ v_aug accumulates unnormalized out^T [65, tq] in PSUM (bf16 matmuls);
    row 64 is the softmax denominator.
  - PE transpose [65,128] -> [128,65], then DVE reciprocal + tensor_scalar mul
    normalizes and writes [t, d] output tiles; DMA to DRAM.
"""

import numpy as np
import ml_dtypes

import concourse.bass as bass
import concourse.mybir as mybir
import concourse.tile as tile
from concourse import bacc
from concourse.bass_utils import run_bass_kernel_spmd

F32 = mybir.dt.float32
F32R = mybir.dt.float32r
BF16 = mybir.dt.bfloat16

B, S, E = 2, 2048, 1024
H, D = 16, 64
NCORES = 8
OC = 256          # output channels per core (4 heads)
NPAIR = 2         # head pairs per core
NT = S // 128     # 16 tk tiles
NT4 = S // 512    # 4 tq blocks

_cached_nc = None


def _build():
    nc = bacc.Bacc()

    hT = nc.declare_dram_parameter("hT", [E, S], BF16, isOutput=False)
    wqT = nc.declare_dram_parameter("wqT", [E, OC], BF16, isOutput=False)
    wkT = nc.declare_dram_parameter("wkT", [E, OC], BF16, isOutput=False)
    wvT = nc.declare_dram_parameter("wvT", [E, OC], BF16, isOutput=False)
    bqp = nc.declare_dram_parameter("bqp", [128, 2], F32, isOutput=False)
    bkp = nc.declare_dram_parameter("bkp", [128, 2], F32, isOutput=False)
    bvf = nc.declare_dram_parameter("bvf", [OC], F32, isOutput=False)
    mask_t = nc.declare_dram_parameter("mask_t", [128, NT], F32, isOutput=False)
    ident = nc.declare_dram_parameter("ident", [65, 65], BF16, isOutput=False)
    out = nc.declare_dram_parameter("out", [S, OC], F32, isOutput=True)

    EXP = mybir.ActivationFunctionType.Exp
    ADD = mybir.AluOpType.add
    MULT = mybir.AluOpType.mult
    GE = mybir.AluOpType.is_ge

    with tile.TileContext(nc) as tc:
        with (
            tc.tile_pool(name="cst", bufs=1) as cst,
            tc.tile_pool(name="work", bufs=3) as work,
            tc.tile_pool(name="expp", bufs=8) as expp,
            tc.tile_pool(name="ps_small", bufs=2, space="PSUM") as ps_small,
            tc.tile_pool(name="ps_sc", bufs=2, space="PSUM") as ps_sc,
            tc.tile_pool(name="ps_out", bufs=2, space="PSUM") as ps_out,
        ):
            # ---- constants ----
            mask_sb = cst.tile([128, NT], F32, tag="mask")
            nc.sync.dma_start(out=mask_sb, in_=mask_t[:, :])
            ident_sb = cst.tile([65, 65], BF16, tag="ident")
            nc.sync.dma_start(out=ident_sb, in_=ident[:, :])
            bq_sb = cst.tile([128, 2], F32, tag="bq")
            nc.sync.dma_start(out=bq_sb, in_=bqp[:, :])
            bk_sb = cst.tile([128, 2], F32, tag="bk")
            nc.sync.dma_start(out=bk_sb, in_=bkp[:, :])
            # bv broadcast to all partitions: [128, 256]
            bv_sb = cst.tile([128, OC], F32, tag="bv")
            nc.gpsimd.dma_start(out=bv_sb, in_=bvf[:].partition_broadcast(128))

            # ---- big resident inputs; hT split [e][t4] and DMA'd in the
            # order the prologue consumes it (q block3, k block0, rest) ----
            hT32 = [[None] * NT4 for _ in range(8)]
            w_sb = {"q": [], "k": [], "v": []}

            def dma_ht(e, t4):
                t = cst.tile([128, 512], BF16, tag=f"hT{e}_{t4}", name=f"hT{e}_{t4}")
                nc.sync.dma_start(
                    out=t, in_=hT[128 * e:128 * (e + 1), 512 * t4:512 * (t4 + 1)])
                hT32[e][t4] = t

            for e in range(8):
                wt = cst.tile([128, OC], BF16, tag=f"wq{e}", name=f"wq{e}")
                nc.sync.dma_start(out=wt, in_=wqT[128 * e:128 * (e + 1), :])
                w_sb["q"].append(wt)
                dma_ht(e, 3)
            for e in range(8):
                wt = cst.tile([128, OC], BF16, tag=f"wk{e}", name=f"wk{e}")
                nc.sync.dma_start(out=wt, in_=wkT[128 * e:128 * (e + 1), :])
                w_sb["k"].append(wt)
                dma_ht(e, 0)
            for e in range(8):
                wt = cst.tile([128, OC], BF16, tag=f"wv{e}", name=f"wv{e}")
                nc.sync.dma_start(out=wt, in_=wvT[128 * e:128 * (e + 1), :])
                w_sb["v"].append(wt)
            for t4 in (1, 2):
                for e in range(8):
                    dma_ht(e, t4)

            # ---- persistent intermediates ----
            qT = [cst.tile([128, S], BF16, tag=f"qT{p}", name=f"qT{p}") for p in range(NPAIR)]
            kT = [cst.tile([128, S], BF16, tag=f"kT{p}", name=f"kT{p}") for p in range(NPAIR)]
            vaug = [[cst.tile([128, 130], BF16, tag=f"va{p}_{tt}", name=f"va{p}_{tt}")
                     for tt in range(NT)] for p in range(NPAIR)]
            outsb = [cst.tile([128, OC], F32, tag=f"o{tt}", name=f"o{tt}") for tt in range(NT)]

            def emit_qk_chain(nm, p, t4):
                po = 128 * p
                dst = qT[p] if nm == "q" else kT[p]
                b_sb = bq_sb if nm == "q" else bk_sb
                ts = slice(512 * t4, 512 * (t4 + 1))
                ps_qk = ps_small.tile([128, 512], F32, tag="sm", name="ps_qk")
                for e in range(8):
                    nc.tensor.matmul(
                        ps_qk,
                        w_sb[nm][e][:, po:po + 128],
                        hT32[e][t4],
                        start=(e == 0), stop=(e == 7),
                    )
                if nm == "q":
                    nc.vector.tensor_scalar(
                        out=dst[:, ts], in0=ps_qk,
                        scalar1=0.125, scalar2=b_sb[:, p:p + 1],
                        op0=MULT, op1=ADD,
                    )
                else:
                    nc.vector.tensor_scalar_add(
                        out=dst[:, ts], in0=ps_qk, scalar1=b_sb[:, p:p + 1],
                    )

            def emit_v_chain(tt):
                t4v, r4 = divmod(tt, 4)
                rs = slice(128 * r4, 128 * (r4 + 1))
                ps_v = ps_small.tile([128, OC], F32, tag="sm", name="ps_v")
                for e in range(8):
                    nc.tensor.matmul(
                        ps_v,
                        hT32[e][t4v][:, rs],
                        w_sb["v"][e][:, :],
                        start=(e == 0), stop=(e == 7),
                    )
                for p in range(NPAIR):
                    po = 128 * p
                    vt = vaug[p][tt]
                    vt3 = vt.rearrange("a (h c) -> a h c", h=2)[:, :, 0:64]
                    ps3 = ps_v[:, po:po + 128].rearrange("a (h c) -> a h c", h=2)
                    bv3 = bv_sb[:, po:po + 128].rearrange("a (h c) -> a h c", h=2)
                    nc.vector.tensor_add(vt3, ps3, bv3)
                    nc.vector.memset(
                        vt.rearrange("a (h c) -> a h c", h=2)[:, :, 64:65], 1.0)

            chores_q = []

            def emit_attn_block(p, j):
                po = 128 * p
                qs = slice(512 * j, 512 * (j + 1))
                out_A = ps_out.tile([65, 512], F32, tag="out", name="out_A")
                out_B = ps_out.tile([65, 512], F32, tag="out", name="out_B")
                ntk = 4 * (j + 1)
                for i in range(ntk):
                    ks = slice(128 * i, 128 * (i + 1))
                    sc = ps_sc.tile([128, 1024], F32, tag="sc", name="sc")
                    nc.tensor.matmul(sc[:, 0:512], kT[p][0:64, ks],
                                     qT[p][0:64, qs], start=True, stop=True)
                    nc.tensor.matmul(sc[:, 512:1024], kT[p][64:128, ks],
                                     qT[p][64:128, qs], start=True, stop=True)
                    ex = expp.tile([128, 1024], BF16, tag="exp", name="ex")
                    nc.scalar.activation(out=ex, in_=sc, func=EXP,
                                         bias=mask_sb[:, i:i + 1], scale=1.0)
                    if i >= 4 * j:
                        s_off = 128 * i - 512 * j
                        ex3 = ex.rearrange("a (h f) -> a h f", h=2)
                        nc.gpsimd.affine_select(
                            out=ex3, in_=ex3, compare_op=GE, fill=0.0,
                            base=-s_off, pattern=[[0, 2], [1, 512]],
                            channel_multiplier=-1,
                        )
                    nc.tensor.matmul(out_A, vaug[p][i][:, 0:65],
                                     ex[:, 0:512],
                                     start=(i == 0), stop=(i == ntk - 1))
                    nc.tensor.matmul(out_B, vaug[p][i][:, 65:130],
                                     ex[:, 512:1024],
                                     start=(i == 0), stop=(i == ntk - 1))
                    if chores_q:
                        chores_q.pop(0)()

                for h_loc, out_ps in ((0, out_A), (1, out_B)):
                    u = work.tile([65, 512], BF16, tag="u", name="u")
                    nc.vector.tensor_copy(u, out_ps)
                    for s4 in range(4):
                        tp = ps_small.tile([128, 65], BF16, tag="sm", name="tp")
                        nc.tensor.transpose(tp, u[:, 128 * s4:128 * (s4 + 1)],
                                            ident_sb)
                        r = work.tile([128, 1], F32, tag="r", name="r")
                        nc.vector.reciprocal(r, tp[:, 64:65])
                        tt = 4 * j + s4
                        c0 = po + 64 * h_loc
                        nc.vector.tensor_scalar_mul(
                            outsb[tt][:, c0:c0 + 64], tp[:, 0:64], r)
                if p == NPAIR - 1:
                    for s4 in range(4):
                        tt = 4 * j + s4
                        nc.sync.dma_start(
                            out=out[128 * tt:128 * (tt + 1), :],
                            in_=outsb[tt])

            # prologue: minimal producers for attention(p0, j=3, i=0..)
            emit_qk_chain("q", 0, 3)
            emit_qk_chain("k", 0, 0)
            for tt in range(4):
                emit_v_chain(tt)
            chores_q.append(lambda: emit_qk_chain("k", 0, 1))
            chores_q.append(lambda: emit_v_chain(4))
            chores_q.append(lambda: emit_v_chain(5))
            chores_q.append(lambda: emit_v_chain(6))
            chores_q.append(lambda: emit_qk_chain("k", 0, 2))
            chores_q.append(lambda: emit_v_chain(7))
            chores_q.append(lambda: emit_v_chain(8))
            chores_q.append(lambda: emit_v_chain(9))
            chores_q.append(lambda: emit_qk_chain("k", 0, 3))
            for _tt in range(10, 16):
                chores_q.append(lambda _tt=_tt: emit_v_chain(_tt))
            chores_q.append(lambda: emit_qk_chain("q", 0, 2))
            chores_q.append(lambda: emit_qk_chain("q", 0, 1))
            chores_q.append(lambda: emit_qk_chain("q", 0, 0))
            for t4 in (3, 2, 1, 0):
                chores_q.append(lambda t4=t4: emit_qk_chain("q", 1, t4))
                chores_q.append(lambda t4=t4: emit_qk_chain("k", 1, 3 - t4))
            # interleave the two pairs' blocks: big blocks feed ACT early,
            # small blocks finish last (short tail)
            for p, j in ((0, 3), (0, 2), (1, 3), (0, 1), (1, 2), (0, 0), (1, 1), (1, 0)):
                emit_attn_block(p, j)

    nc.compile()
    return nc


def _get_nc():
    global _cached_nc
    if _cached_nc is None:
        _cached_nc = _build()
    return _cached_nc


def kernel(hidden_states, attention_mask, Wq, bq, Wk, bk, Wv, bv):
    hidden_states = np.asarray(hidden_states, dtype=np.float32)
    attention_mask = np.asarray(attention_mask, dtype=np.float32)
    Wq = np.asarray(Wq, dtype=np.float32)
    Wk = np.asarray(Wk, dtype=np.float32)
    Wv = np.asarray(Wv, dtype=np.float32)
    bq = np.asarray(bq, dtype=np.float32)
    bk = np.asarray(bk, dtype=np.float32)
    bv = np.asarray(bv, dtype=np.float32)

    bf = ml_dtypes.bfloat16
    ident = np.eye(65, dtype=np.float32).astype(ml_dtypes.bfloat16)
    in_maps = []
    for c in range(NCORES):
        b, g = divmod(c, 4)
        cs = slice(OC * g, OC * (g + 1))
        in_maps.append({
            "hT": np.ascontiguousarray(hidden_states[b].T).astype(bf),
            "wqT": np.ascontiguousarray(Wq[cs, :].T).astype(bf),
            "wkT": np.ascontiguousarray(Wk[cs, :].T).astype(bf),
            "wvT": np.ascontiguousarray(Wv[cs, :].T).astype(bf),
            "bqp": np.ascontiguousarray(bq[cs].reshape(2, 128).T),
            "bkp": np.ascontiguousarray(bk[cs].reshape(2, 128).T),
            "bvf": np.ascontiguousarray(bv[cs]),
            "mask_t": np.ascontiguousarray(
                attention_mask[b, 0, 0, :].reshape(NT, 128).T),
            "ident": ident,
        })

    nc = _get_nc()
    res = run_bass_kernel_spmd(nc, in_maps, list(range(NCORES)))

    full = np.empty((B, S, H * D), dtype=np.float32)
    for c in range(NCORES):
        b, g = divmod(c, 4)
        full[b, :, OC * g:OC * (g + 1)] = res.results[c]["out"]
    return full


# revision 9
# speedup vs baseline: 1.0504x; 1.0127x over previous
"""Causal self-attention (B=2, S=2048, E=1024, H=16, D=64) on 8 trn2 NeuronCores.

Sharding: core c = (batch b = c // 4, head-group g = c % 4).  Each core computes
4 heads (one quarter of the 16) for one batch: projections q/k/v for its 256
output channels, then causal flash-style attention, writing out[b, :, 256g:256g+256].

Per-core kernel design (Bass/Tile):
  - Host pre-transposes hidden -> hT [E, S] (bf16) and weight slices -> wT [E, 256]
    (bf16) so all matmul contractions have K on partitions.
  - q/k projections (bf16, PSUM-accumulated over 8 E-chunks) produce qT/kT in
    [d, t] layout, copied to SBUF as float32r with scale 1/8 (q) and bias add.
  - v projection produces v in [t, d] layout; DVE copy splits heads into
    v_aug tiles [tk=128, 65*2] with a ones column per head (sum-of-exp trick).
  - scores^T tiles [tk=128, tq=512] per head via single f32r matmuls; the two
    heads of a pair run concurrently on PE row halves (K=64 each).
  - exp via ScalarE activation (attention-mask bias per tk partition), bf16 out.
  - causal masking: gpsimd affine_select zeroes the invalid region of
    diagonal-crossing tiles after exp.
  - attn @ v_aug accumulates unnormalized out^T [65, tq] in PSUM (bf16 matmuls);
    row 64 is the softmax denominator.
  - PE transpose [65,128] -> [128,65], then DVE reciprocal + tensor_scalar mul
    normalizes and writes [t, d] output tiles; DMA to DRAM.
"""

import numpy as np
import ml_dtypes

import concourse.bass as bass
import concourse.mybir as mybir
import concourse.tile as tile
from concourse import bacc
from concourse.bass_utils import run_bass_kernel_spmd

F32 = mybir.dt.float32
F32R = mybir.dt.float32r
BF16 = mybir.dt.bfloat16

B, S, E = 2, 2048, 1024
H, D = 16, 64
NCORES = 8
OC = 256          # output channels per core (4 heads)
NPAIR = 2         # head pairs per core
NT = S // 128     # 16 tk tiles
NT4 = S // 512    # 4 tq blocks

_cached_nc = None


def _build():
    nc = bacc.Bacc()

    hT = nc.declare_dram_parameter("hT", [E, S], BF16, isOutput=False)
    wqT = nc.declare_dram_parameter("wqT", [E, OC], BF16, isOutput=False)
    wkT = nc.declare_dram_parameter("wkT", [E, OC], BF16, isOutput=False)
    wvT = nc.declare_dram_parameter("wvT", [E, OC], BF16, isOutput=False)
    bqp = nc.declare_dram_parameter("bqp", [128, 2], F32, isOutput=False)
    bkp = nc.declare_dram_parameter("bkp", [128, 2], F32, isOutput=False)
    bvf = nc.declare_dram_parameter("bvf", [OC], F32, isOutput=False)
    mask_t = nc.declare_dram_parameter("mask_t", [128, NT], F32, isOutput=False)
    ident = nc.declare_dram_parameter("ident", [65, 65], BF16, isOutput=False)
    out = nc.declare_dram_parameter("out", [S, OC], F32, isOutput=True)

    EXP = mybir.ActivationFunctionType.Exp
    ADD = mybir.AluOpType.add
    MULT = mybir.AluOpType.mult
    GE = mybir.AluOpType.is_ge

    with tile.TileContext(nc) as tc:
        with (
            tc.tile_pool(name="cst", bufs=1) as cst,
            tc.tile_pool(name="work", bufs=3) as work,
            tc.tile_pool(name="expp", bufs=8) as expp,
            tc.tile_pool(name="ps_small", bufs=2, space="PSUM") as ps_small,
            tc.tile_pool(name="ps_sc", bufs=2, space="PSUM") as ps_sc,
            tc.tile_pool(name="ps_out", bufs=2, space="PSUM") as ps_out,
        ):
            # ---- constants ----
            mask_sb = cst.tile([128, NT], F32, tag="mask")
            nc.sync.dma_start(out=mask_sb, in_=mask_t[:, :])
            ident_sb = cst.tile([65, 65], BF16, tag="ident")
            nc.sync.dma_start(out=ident_sb, in_=ident[:, :])
            bq_sb = cst.tile([128, 2], F32, tag="bq")
            nc.sync.dma_start(out=bq_sb, in_=bqp[:, :])
            bk_sb = cst.tile([128, 2], F32, tag="bk")
            nc.sync.dma_start(out=bk_sb, in_=bkp[:, :])
            # bv broadcast to all partitions: [128, 256]
            bv_sb = cst.tile([128, OC], F32, tag="bv")
            nc.gpsimd.dma_start(out=bv_sb, in_=bvf[:].partition_broadcast(128))

            # ---- big resident inputs; hT split [e][t4] and DMA'd in the
            # order the prologue consumes it (q block3, k block0, rest) ----
            hT32 = [[None] * NT4 for _ in range(8)]
            w_sb = {"q": [], "k": [], "v": []}

            def dma_ht(e, t4):
                t = cst.tile([128, 512], BF16, tag=f"hT{e}_{t4}", name=f"hT{e}_{t4}")
                nc.sync.dma_start(
                    out=t, in_=hT[128 * e:128 * (e + 1), 512 * t4:512 * (t4 + 1)])
                hT32[e][t4] = t

            for e in range(8):
                wt = cst.tile([128, OC], BF16, tag=f"wq{e}", name=f"wq{e}")
                nc.sync.dma_start(out=wt, in_=wqT[128 * e:128 * (e + 1), :])
                w_sb["q"].append(wt)
                dma_ht(e, 3)
            for e in range(8):
                wt = cst.tile([128, OC], BF16, tag=f"wk{e}", name=f"wk{e}")
                nc.sync.dma_start(out=wt, in_=wkT[128 * e:128 * (e + 1), :])
                w_sb["k"].append(wt)
                dma_ht(e, 0)
            for e in range(8):
                wt = cst.tile([128, OC], BF16, tag=f"wv{e}", name=f"wv{e}")
                nc.sync.dma_start(out=wt, in_=wvT[128 * e:128 * (e + 1), :])
                w_sb["v"].append(wt)
            for t4 in (1, 2):
                for e in range(8):
                    dma_ht(e, t4)

            # ---- persistent intermediates ----
            qT = [cst.tile([128, S], BF16, tag=f"qT{p}", name=f"qT{p}") for p in range(NPAIR)]
            kT = [cst.tile([128, S], BF16, tag=f"kT{p}", name=f"kT{p}") for p in range(NPAIR)]
            vaug = [[cst.tile([128, 130], BF16, tag=f"va{p}_{tt}", name=f"va{p}_{tt}")
                     for tt in range(NT)] for p in range(NPAIR)]
            outsb = [cst.tile([128, OC], F32, tag=f"o{tt}", name=f"o{tt}") for tt in range(NT)]

            def emit_qk_chain(nm, p, t4):
                po = 128 * p
                dst = qT[p] if nm == "q" else kT[p]
                b_sb = bq_sb if nm == "q" else bk_sb
                ts = slice(512 * t4, 512 * (t4 + 1))
                ps_qk = ps_small.tile([128, 512], F32, tag="sm", name="ps_qk")
                for e in range(8):
                    nc.tensor.matmul(
                        ps_qk,
                        w_sb[nm][e][:, po:po + 128],
                        hT32[e][t4],
                        start=(e == 0), stop=(e == 7),
                    )
                if nm == "q":
                    nc.vector.tensor_scalar(
                        out=dst[:, ts], in0=ps_qk,
                        scalar1=0.125, scalar2=b_sb[:, p:p + 1],
                        op0=MULT, op1=ADD,
                    )
                else:
                    nc.vector.tensor_scalar_add(
                        out=dst[:, ts], in0=ps_qk, scalar1=b_sb[:, p:p + 1],
                    )

            def emit_v_chain(tt):
                t4v, r4 = divmod(tt, 4)
                rs = slice(128 * r4, 128 * (r4 + 1))
                ps_v = ps_small.tile([128, OC], F32, tag="sm", name="ps_v")
                for e in range(8):
                    nc.tensor.matmul(
                        ps_v,
                        hT32[e][t4v][:, rs],
                        w_sb["v"][e][:, :],
                        start=(e == 0), stop=(e == 7),
                    )
                for p in range(NPAIR):
                    po = 128 * p
                    vt = vaug[p][tt]
                    vt3 = vt.rearrange("a (h c) -> a h c", h=2)[:, :, 0:64]
                    ps3 = ps_v[:, po:po + 128].rearrange("a (h c) -> a h c", h=2)
                    bv3 = bv_sb[:, po:po + 128].rearrange("a (h c) -> a h c", h=2)
                    nc.vector.tensor_add(vt3, ps3, bv3)
                    nc.vector.memset(
                        vt.rearrange("a (h c) -> a h c", h=2)[:, :, 64:65], 1.0)

            chores_q = []

            def emit_attn_block(p, j):
                po = 128 * p
                qs = slice(512 * j, 512 * (j + 1))
                out_A = ps_out.tile([65, 512], F32, tag="out", name="out_A")
                out_B = ps_out.tile([65, 512], F32, tag="out", name="out_B")
                ntk = 4 * (j + 1)
                for i in range(ntk):
                    ks = slice(128 * i, 128 * (i + 1))
                    crossing = i >= 4 * j
                    # valid tq columns of this tile start at s (cols < s are
                    # entirely above the diagonal): trim scores/exp/attn@v
                    s = 128 * i - 512 * j if crossing else 0
                    qsv = slice(512 * j + s, 512 * (j + 1))
                    sc = ps_sc.tile([128, 1024], F32, tag="sc", name="sc")
                    nc.tensor.matmul(sc[:, s:512], kT[p][0:64, ks],
                                     qT[p][0:64, qsv], start=True, stop=True)
                    nc.tensor.matmul(sc[:, 512 + s:1024], kT[p][64:128, ks],
                                     qT[p][64:128, qsv], start=True, stop=True)
                    ex = expp.tile([128, 1024], BF16, tag="exp", name="ex")
                    if s:
                        exv = ex.rearrange("a (h f) -> a h f", h=2)[:, :, s:512]
                        scv = sc.rearrange("a (h f) -> a h f", h=2)[:, :, s:512]
                    else:
                        exv, scv = ex, sc
                    nc.scalar.activation(out=exv, in_=scv, func=EXP,
                                         bias=mask_sb[:, i:i + 1], scale=1.0)
                    if crossing:
                        s_off = s
                        ex3 = ex.rearrange("a (h f) -> a h f", h=2)
                        nc.gpsimd.affine_select(
                            out=ex3, in_=ex3, compare_op=GE, fill=0.0,
                            base=-s_off, pattern=[[0, 2], [1, 512]],
                            channel_multiplier=-1,
                        )
                    nc.tensor.matmul(out_A[:, s:512], vaug[p][i][:, 0:65],
                                     ex[:, s:512],
                                     start=(i == 0), stop=(i == ntk - 1))
                    nc.tensor.matmul(out_B[:, s:512], vaug[p][i][:, 65:130],
                                     ex[:, 512 + s:1024],
                                     start=(i == 0), stop=(i == ntk - 1))
                    if chores_q:
                        chores_q.pop(0)()

                for h_loc, out_ps in ((0, out_A), (1, out_B)):
                    u = work.tile([65, 512], BF16, tag="u", name="u")
                    nc.vector.tensor_copy(u, out_ps)
                    for s4 in range(4):
                        tp = ps_small.tile([128, 65], BF16, tag="sm", name="tp")
                        nc.tensor.transpose(tp, u[:, 128 * s4:128 * (s4 + 1)],
                                            ident_sb)
                        r = work.tile([128, 1], F32, tag="r", name="r")
                        nc.vector.reciprocal(r, tp[:, 64:65])
                        tt = 4 * j + s4
                        c0 = po + 64 * h_loc
                        nc.vector.tensor_scalar_mul(
                            outsb[tt][:, c0:c0 + 64], tp[:, 0:64], r)
                if p == NPAIR - 1:
                    for s4 in range(4):
                        tt = 4 * j + s4
                        nc.sync.dma_start(
                            out=out[128 * tt:128 * (tt + 1), :],
                            in_=outsb[tt])

            # prologue: minimal producers for attention(p0, j=3, i=0..)
            emit_qk_chain("q", 0, 3)
            emit_qk_chain("k", 0, 0)
            for tt in range(4):
                emit_v_chain(tt)
            chores_q.append(lambda: emit_qk_chain("k", 0, 1))
            chores_q.append(lambda: emit_v_chain(4))
            chores_q.append(lambda: emit_v_chain(5))
            chores_q.append(lambda: emit_v_chain(6))
            chores_q.append(lambda: emit_qk_chain("k", 0, 2))
            chores_q.append(lambda: emit_v_chain(7))
            chores_q.append(lambda: emit_v_chain(8))
            chores_q.append(lambda: emit_v_chain(9))
            chores_q.append(lambda: emit_qk_chain("k", 0, 3))
            for _tt in range(10, 16):
                chores_q.append(lambda _tt=_tt: emit_v_chain(_tt))
            chores_q.append(lambda: emit_qk_chain("q", 0, 2))
            chores_q.append(lambda: emit_qk_chain("q", 0, 1))
            chores_q.append(lambda: emit_qk_chain("q", 0, 0))
            for t4 in (3, 2, 1, 0):
                chores_q.append(lambda t4=t4: emit_qk_chain("q", 1, t4))
                chores_q.append(lambda t4=t4: emit_qk_chain("k", 1, 3 - t4))
            # interleave the two pairs' blocks: big blocks feed ACT early,
            # small blocks finish last (short tail)
            for p, j in ((0, 3), (0, 2), (1, 3), (0, 1), (1, 2), (0, 0), (1, 1), (1, 0)):
                emit_attn_block(p, j)

    nc.compile()
    return nc


def _get_nc():
    global _cached_nc
    if _cached_nc is None:
        _cached_nc = _build()
    return _cached_nc


def kernel(hidden_states, attention_mask, Wq, bq, Wk, bk, Wv, bv):
    hidden_states = np.asarray(hidden_states, dtype=np.float32)
    attention_mask = np.asarray(attention_mask, dtype=np.float32)
    Wq = np.asarray(Wq, dtype=np.float32)
    Wk = np.asarray(Wk, dtype=np.float32)
    Wv = np.asarray(Wv, dtype=np.float32)
    bq = np.asarray(bq, dtype=np.float32)
    bk = np.asarray(bk, dtype=np.float32)
    bv = np.asarray(bv, dtype=np.float32)

    bf = ml_dtypes.bfloat16
    ident = np.eye(65, dtype=np.float32).astype(ml_dtypes.bfloat16)
    in_maps = []
    for c in range(NCORES):
        b, g = divmod(c, 4)
        cs = slice(OC * g, OC * (g + 1))
        in_maps.append({
            "hT": np.ascontiguousarray(hidden_states[b].T).astype(bf),
            "wqT": np.ascontiguousarray(Wq[cs, :].T).astype(bf),
            "wkT": np.ascontiguousarray(Wk[cs, :].T).astype(bf),
            "wvT": np.ascontiguousarray(Wv[cs, :].T).astype(bf),
            "bqp": np.ascontiguousarray(bq[cs].reshape(2, 128).T),
            "bkp": np.ascontiguousarray(bk[cs].reshape(2, 128).T),
            "bvf": np.ascontiguousarray(bv[cs]),
            "mask_t": np.ascontiguousarray(
                attention_mask[b, 0, 0, :].reshape(NT, 128).T),
            "ident": ident,
        })

    nc = _get_nc()
    res = run_bass_kernel_spmd(nc, in_maps, list(range(NCORES)))

    full = np.empty((B, S, H * D), dtype=np.float32)
    for c in range(NCORES):
        b, g = divmod(c, 4)
        full[b, :, OC * g:OC * (g + 1)] = res.results[c]["out"]
    return full


# revision 10
# speedup vs baseline: 1.1095x; 1.0563x over previous
"""Causal self-attention (B=2, S=2048, E=1024, H=16, D=64) on 8 trn2 NeuronCores.

Sharding: core c = (batch b = c // 4, head-group g = c % 4).  Each core computes
4 heads (one quarter of the 16) for one batch: projections q/k/v for its 256
output channels, then causal flash-style attention, writing out[b, :, 256g:256g+256].

Per-core kernel design (Bass/Tile):
  - Host pre-transposes hidden -> hT [E, S] (bf16) and weight slices -> wT [E, 256]
    (bf16) so all matmul contractions have K on partitions.
  - q/k projections (bf16, PSUM-accumulated over 8 E-chunks) produce qT/kT in
    [d, t] layout, copied to SBUF as float32r with scale 1/8 (q) and bias add.
  - v projection produces v in [t, d] layout; DVE copy splits heads into
    v_aug tiles [tk=128, 65*2] with a ones column per head (sum-of-exp trick).
  - scores^T tiles [tk=128, tq=512] per head via single f32r matmuls; the two
    heads of a pair run concurrently on PE row halves (K=64 each).
  - exp via ScalarE activation (attention-mask bias per tk partition), bf16 out.
  - causal masking: gpsimd affine_select zeroes the invalid region of
    diagonal-crossing tiles after exp.
  - attn @ v_aug accumulates unnormalized out^T [65, tq] in PSUM (bf16 matmuls);
    row 64 is the softmax denominator.
  - PE transpose [65,128] -> [128,65], then DVE reciprocal + tensor_scalar mul
    normalizes and writes [t, d] output tiles; DMA to DRAM.
"""

import numpy as np
import ml_dtypes

import concourse.bass as bass
import concourse.mybir as mybir
import concourse.tile as tile
from concourse import bacc
from concourse.bass_utils import run_bass_kernel_spmd

F32 = mybir.dt.float32
F32R = mybir.dt.float32r
BF16 = mybir.dt.bfloat16

B, S, E = 2, 2048, 1024
H, D = 16, 64
NCORES = 8
OC = 256          # output channels per core (4 heads)
NPAIR = 2         # head pairs per core
NT = S // 128     # 16 tk tiles
NT4 = S // 512    # 4 tq blocks

_cached_nc = None


def _build():
    nc = bacc.Bacc()

    hT = nc.declare_dram_parameter("hT", [128, 32 * 512], BF16, isOutput=False)
    wqT = nc.declare_dram_parameter("wqT", [128, 2048], BF16, isOutput=False)
    wkT = nc.declare_dram_parameter("wkT", [128, 2048], BF16, isOutput=False)
    wvT = nc.declare_dram_parameter("wvT", [128, 2048], BF16, isOutput=False)
    bqp = nc.declare_dram_parameter("bqp", [128, 2], F32, isOutput=False)
    bkp = nc.declare_dram_parameter("bkp", [128, 2], F32, isOutput=False)
    bvf = nc.declare_dram_parameter("bvf", [OC], F32, isOutput=False)
    mask_t = nc.declare_dram_parameter("mask_t", [128, NT], F32, isOutput=False)
    ident = nc.declare_dram_parameter("ident", [65, 65], BF16, isOutput=False)
    out = nc.declare_dram_parameter("out", [S, OC], F32, isOutput=True)

    EXP = mybir.ActivationFunctionType.Exp
    ADD = mybir.AluOpType.add
    MULT = mybir.AluOpType.mult
    GE = mybir.AluOpType.is_ge

    with tile.TileContext(nc) as tc:
        with (
            tc.tile_pool(name="cst", bufs=1) as cst,
            tc.tile_pool(name="work", bufs=3) as work,
            tc.tile_pool(name="expp", bufs=8) as expp,
            tc.tile_pool(name="ps_small", bufs=2, space="PSUM") as ps_small,
            tc.tile_pool(name="ps_sc", bufs=2, space="PSUM") as ps_sc,
            tc.tile_pool(name="ps_out", bufs=2, space="PSUM") as ps_out,
        ):
            # ---- constants ----
            mask_sb = cst.tile([128, NT], F32, tag="mask")
            nc.sync.dma_start(out=mask_sb, in_=mask_t[:, :])
            ident_sb = cst.tile([65, 65], BF16, tag="ident")
            nc.sync.dma_start(out=ident_sb, in_=ident[:, :])
            bq_sb = cst.tile([128, 2], F32, tag="bq")
            nc.sync.dma_start(out=bq_sb, in_=bqp[:, :])
            bk_sb = cst.tile([128, 2], F32, tag="bk")
            nc.sync.dma_start(out=bk_sb, in_=bkp[:, :])
            # bv broadcast to all partitions: [128, 256]
            bv_sb = cst.tile([128, OC], F32, tag="bv")
            nc.gpsimd.dma_start(out=bv_sb, in_=bvf[:].partition_broadcast(128))

            # ---- big resident inputs: few large DMAs, host-packed in
            # consumption order (t4 groups 3,0,1,2; e-chunks side by side) ----
            G = {3: 0, 0: 1, 1: 2, 2: 3}  # t4 -> group position
            hT_big = cst.tile([128, 32 * 512], BF16, tag="hT_big")
            wq_big = cst.tile([128, 2048], BF16, tag="wq_big")
            nc.sync.dma_start(out=wq_big, in_=wqT[:, :])
            nc.sync.dma_start(out=hT_big[:, 0:4096], in_=hT[:, 0:4096])
            wk_big = cst.tile([128, 2048], BF16, tag="wk_big")
            nc.sync.dma_start(out=wk_big, in_=wkT[:, :])
            nc.sync.dma_start(out=hT_big[:, 4096:8192], in_=hT[:, 4096:8192])
            wv_big = cst.tile([128, 2048], BF16, tag="wv_big")
            nc.sync.dma_start(out=wv_big, in_=wvT[:, :])
            nc.sync.dma_start(out=hT_big[:, 8192:12288], in_=hT[:, 8192:12288])
            nc.sync.dma_start(out=hT_big[:, 12288:16384], in_=hT[:, 12288:16384])

            hT32 = [[hT_big[:, G[t4] * 4096 + e * 512: G[t4] * 4096 + (e + 1) * 512]
                     for t4 in range(NT4)] for e in range(8)]
            w_sb = {nm: [big[:, e * OC:(e + 1) * OC] for e in range(8)]
                    for nm, big in (("q", wq_big), ("k", wk_big), ("v", wv_big))}

            # ---- persistent intermediates ----
            qT = [cst.tile([128, S], BF16, tag=f"qT{p}", name=f"qT{p}") for p in range(NPAIR)]
            kT = [cst.tile([128, S], BF16, tag=f"kT{p}", name=f"kT{p}") for p in range(NPAIR)]
            vaug = [[cst.tile([128, 130], BF16, tag=f"va{p}_{tt}", name=f"va{p}_{tt}")
                     for tt in range(NT)] for p in range(NPAIR)]
            outsb = [cst.tile([128, OC], F32, tag=f"o{tt}", name=f"o{tt}") for tt in range(NT)]

            def emit_qk_chain(nm, p, t4):
                po = 128 * p
                dst = qT[p] if nm == "q" else kT[p]
                b_sb = bq_sb if nm == "q" else bk_sb
                ts = slice(512 * t4, 512 * (t4 + 1))
                ps_qk = ps_small.tile([128, 512], F32, tag="sm", name="ps_qk")
                for e in range(8):
                    nc.tensor.matmul(
                        ps_qk,
                        w_sb[nm][e][:, po:po + 128],
                        hT32[e][t4],
                        start=(e == 0), stop=(e == 7),
                    )
                if nm == "q":
                    nc.vector.tensor_scalar(
                        out=dst[:, ts], in0=ps_qk,
                        scalar1=0.125, scalar2=b_sb[:, p:p + 1],
                        op0=MULT, op1=ADD,
                    )
                else:
                    nc.vector.tensor_scalar_add(
                        out=dst[:, ts], in0=ps_qk, scalar1=b_sb[:, p:p + 1],
                    )

            def emit_v_chain(tt):
                t4v, r4 = divmod(tt, 4)
                rs = slice(128 * r4, 128 * (r4 + 1))
                ps_v = ps_small.tile([128, OC], F32, tag="sm", name="ps_v")
                for e in range(8):
                    nc.tensor.matmul(
                        ps_v,
                        hT32[e][t4v][:, rs],
                        w_sb["v"][e][:, :],
                        start=(e == 0), stop=(e == 7),
                    )
                for p in range(NPAIR):
                    po = 128 * p
                    vt = vaug[p][tt]
                    vt3 = vt.rearrange("a (h c) -> a h c", h=2)[:, :, 0:64]
                    ps3 = ps_v[:, po:po + 128].rearrange("a (h c) -> a h c", h=2)
                    bv3 = bv_sb[:, po:po + 128].rearrange("a (h c) -> a h c", h=2)
                    nc.vector.tensor_add(vt3, ps3, bv3)
                    nc.vector.memset(
                        vt.rearrange("a (h c) -> a h c", h=2)[:, :, 64:65], 1.0)

            chores_q = []

            def emit_attn_block(p, j):
                po = 128 * p
                qs = slice(512 * j, 512 * (j + 1))
                out_A = ps_out.tile([65, 512], F32, tag="out", name="out_A")
                out_B = ps_out.tile([65, 512], F32, tag="out", name="out_B")
                ntk = 4 * (j + 1)
                for i in range(ntk):
                    ks = slice(128 * i, 128 * (i + 1))
                    crossing = i >= 4 * j
                    # valid tq columns of this tile start at s (cols < s are
                    # entirely above the diagonal): trim scores/exp/attn@v
                    s = 128 * i - 512 * j if crossing else 0
                    qsv = slice(512 * j + s, 512 * (j + 1))
                    sc = ps_sc.tile([128, 1024], F32, tag="sc", name="sc")
                    nc.tensor.matmul(sc[:, s:512], kT[p][0:64, ks],
                                     qT[p][0:64, qsv], start=True, stop=True)
                    nc.tensor.matmul(sc[:, 512 + s:1024], kT[p][64:128, ks],
                                     qT[p][64:128, qsv], start=True, stop=True)
                    ex = expp.tile([128, 1024], BF16, tag="exp", name="ex")
                    if s:
                        exv = ex.rearrange("a (h f) -> a h f", h=2)[:, :, s:512]
                        scv = sc.rearrange("a (h f) -> a h f", h=2)[:, :, s:512]
                    else:
                        exv, scv = ex, sc
                    nc.scalar.activation(out=exv, in_=scv, func=EXP,
                                         bias=mask_sb[:, i:i + 1], scale=1.0)
                    if crossing:
                        s_off = s
                        ex3 = ex.rearrange("a (h f) -> a h f", h=2)
                        nc.gpsimd.affine_select(
                            out=ex3, in_=ex3, compare_op=GE, fill=0.0,
                            base=-s_off, pattern=[[0, 2], [1, 512]],
                            channel_multiplier=-1,
                        )
                    nc.tensor.matmul(out_A[:, s:512], vaug[p][i][:, 0:65],
                                     ex[:, s:512],
                                     start=(i == 0), stop=(i == ntk - 1))
                    nc.tensor.matmul(out_B[:, s:512], vaug[p][i][:, 65:130],
                                     ex[:, 512 + s:1024],
                                     start=(i == 0), stop=(i == ntk - 1))
                    if chores_q:
                        chores_q.pop(0)()

                for h_loc, out_ps in ((0, out_A), (1, out_B)):
                    u = work.tile([65, 512], BF16, tag="u", name="u")
                    nc.vector.tensor_copy(u, out_ps)
                    for s4 in range(4):
                        tp = ps_small.tile([128, 65], BF16, tag="sm", name="tp")
                        nc.tensor.transpose(tp, u[:, 128 * s4:128 * (s4 + 1)],
                                            ident_sb)
                        r = work.tile([128, 1], F32, tag="r", name="r")
                        nc.vector.reciprocal(r, tp[:, 64:65])
                        tt = 4 * j + s4
                        c0 = po + 64 * h_loc
                        nc.vector.tensor_scalar_mul(
                            outsb[tt][:, c0:c0 + 64], tp[:, 0:64], r)
                if p == NPAIR - 1:
                    for s4 in range(4):
                        tt = 4 * j + s4
                        nc.sync.dma_start(
                            out=out[128 * tt:128 * (tt + 1), :],
                            in_=outsb[tt])

            # prologue: minimal producers for attention(p0, j=3, i=0..)
            emit_qk_chain("q", 0, 3)
            emit_qk_chain("k", 0, 0)
            for tt in range(4):
                emit_v_chain(tt)
            chores_q.append(lambda: emit_qk_chain("k", 0, 1))
            chores_q.append(lambda: emit_v_chain(4))
            chores_q.append(lambda: emit_v_chain(5))
            chores_q.append(lambda: emit_v_chain(6))
            chores_q.append(lambda: emit_qk_chain("k", 0, 2))
            chores_q.append(lambda: emit_v_chain(7))
            chores_q.append(lambda: emit_v_chain(8))
            chores_q.append(lambda: emit_v_chain(9))
            chores_q.append(lambda: emit_qk_chain("k", 0, 3))
            for _tt in range(10, 16):
                chores_q.append(lambda _tt=_tt: emit_v_chain(_tt))
            chores_q.append(lambda: emit_qk_chain("q", 0, 2))
            chores_q.append(lambda: emit_qk_chain("q", 0, 1))
            chores_q.append(lambda: emit_qk_chain("q", 0, 0))
            for t4 in (3, 2, 1, 0):
                chores_q.append(lambda t4=t4: emit_qk_chain("q", 1, t4))
                chores_q.append(lambda t4=t4: emit_qk_chain("k", 1, 3 - t4))
            # interleave the two pairs' blocks: big blocks feed ACT early,
            # small blocks finish last (short tail)
            for p, j in ((0, 3), (0, 2), (1, 3), (0, 1), (1, 2), (0, 0), (1, 1), (1, 0)):
                emit_attn_block(p, j)

    nc.compile()
    return nc


def _get_nc():
    global _cached_nc
    if _cached_nc is None:
        _cached_nc = _build()
    return _cached_nc


def make_in_maps(hidden_states, attention_mask, Wq, bq, Wk, bk, Wv, bv):
    hidden_states = np.asarray(hidden_states, dtype=np.float32)
    attention_mask = np.asarray(attention_mask, dtype=np.float32)
    Wq = np.asarray(Wq, dtype=np.float32)
    Wk = np.asarray(Wk, dtype=np.float32)
    Wv = np.asarray(Wv, dtype=np.float32)
    bq = np.asarray(bq, dtype=np.float32)
    bk = np.asarray(bk, dtype=np.float32)
    bv = np.asarray(bv, dtype=np.float32)

    bf = ml_dtypes.bfloat16
    ident = np.eye(65, dtype=np.float32).astype(bf)
    in_maps = []
    for c in range(NCORES):
        b, g = divmod(c, 4)
        cs = slice(OC * g, OC * (g + 1))
        hTT = np.ascontiguousarray(hidden_states[b].T).astype(bf)  # [E, S]
        hp = np.empty((128, 32 * 512), dtype=bf)
        for gi, t4 in enumerate((3, 0, 1, 2)):
            for e in range(8):
                hp[:, gi * 4096 + e * 512:gi * 4096 + (e + 1) * 512] = \
                    hTT[e * 128:(e + 1) * 128, t4 * 512:(t4 + 1) * 512]

        def packw(W):
            wT = np.ascontiguousarray(W[cs, :].T).astype(bf)  # [E, 256]
            wp = np.empty((128, 2048), dtype=bf)
            for e in range(8):
                wp[:, e * OC:(e + 1) * OC] = wT[e * 128:(e + 1) * 128, :]
            return wp

        in_maps.append({
            "hT": hp,
            "wqT": packw(Wq),
            "wkT": packw(Wk),
            "wvT": packw(Wv),
            "bqp": np.ascontiguousarray(bq[cs].reshape(2, 128).T),
            "bkp": np.ascontiguousarray(bk[cs].reshape(2, 128).T),
            "bvf": np.ascontiguousarray(bv[cs]),
            "mask_t": np.ascontiguousarray(
                attention_mask[b, 0, 0, :].reshape(NT, 128).T),
            "ident": ident,
        })
    return in_maps


def kernel(hidden_states, attention_mask, Wq, bq, Wk, bk, Wv, bv):
    in_maps = make_in_maps(hidden_states, attention_mask,
                           Wq, bq, Wk, bk, Wv, bv)
    nc = _get_nc()
    res = run_bass_kernel_spmd(nc, in_maps, list(range(NCORES)))

    full = np.empty((B, S, H * D), dtype=np.float32)
    for c in range(NCORES):
        b, g = divmod(c, 4)
        full[b, :, OC * g:OC * (g + 1)] = res.results[c]["out"]
    return full


# revision 11
# speedup vs baseline: 1.1336x; 1.0217x over previous
"""Causal self-attention (B=2, S=2048, E=1024, H=16, D=64) on 8 trn2 NeuronCores.

Sharding: core c = (batch b = c // 4, head-group g = c % 4).  Each core computes
4 heads (one quarter of the 16) for one batch: projections q/k/v for its 256
output channels, then causal flash-style attention, writing out[b, :, 256g:256g+256].

Per-core kernel design (Bass/Tile):
  - Host pre-transposes hidden -> hT [E, S] (bf16) and weight slices -> wT [E, 256]
    (bf16) so all matmul contractions have K on partitions.
  - q/k projections (bf16, PSUM-accumulated over 8 E-chunks) produce qT/kT in
    [d, t] layout, copied to SBUF as float32r with scale 1/8 (q) and bias add.
  - v projection produces v in [t, d] layout; DVE copy splits heads into
    v_aug tiles [tk=128, 65*2] with a ones column per head (sum-of-exp trick).
  - scores^T tiles [tk=128, tq=512] per head via single f32r matmuls; the two
    heads of a pair run concurrently on PE row halves (K=64 each).
  - exp via ScalarE activation (attention-mask bias per tk partition), bf16 out.
  - causal masking: gpsimd affine_select zeroes the invalid region of
    diagonal-crossing tiles after exp.
  - attn @ v_aug accumulates unnormalized out^T [65, tq] in PSUM (bf16 matmuls);
    row 64 is the softmax denominator.
  - PE transpose [65,128] -> [128,65], then DVE reciprocal + tensor_scalar mul
    normalizes and writes [t, d] output tiles; DMA to DRAM.
"""

import numpy as np
import ml_dtypes

import concourse.bass as bass
import concourse.mybir as mybir
import concourse.tile as tile
from concourse import bacc
from concourse.bass_utils import run_bass_kernel_spmd

F32 = mybir.dt.float32
F32R = mybir.dt.float32r
BF16 = mybir.dt.bfloat16

B, S, E = 2, 2048, 1024
H, D = 16, 64
NCORES = 8
OC = 256          # output channels per core (4 heads)
NPAIR = 2         # head pairs per core
NT = S // 128     # 16 tk tiles
NT4 = S // 512    # 4 tq blocks

_cached_nc = None


def _build():
    nc = bacc.Bacc()

    hT = nc.declare_dram_parameter("hT", [128, 32 * 512], BF16, isOutput=False)
    wqT = nc.declare_dram_parameter("wqT", [128, 2048], BF16, isOutput=False)
    wkT = nc.declare_dram_parameter("wkT", [128, 2048], BF16, isOutput=False)
    wvT = nc.declare_dram_parameter("wvT", [128, 2048], BF16, isOutput=False)
    bqp = nc.declare_dram_parameter("bqp", [128, 2], F32, isOutput=False)
    bkp = nc.declare_dram_parameter("bkp", [128, 2], F32, isOutput=False)
    bvf = nc.declare_dram_parameter("bvf", [OC], F32, isOutput=False)
    mask_t = nc.declare_dram_parameter("mask_t", [128, NT], F32, isOutput=False)
    ident = nc.declare_dram_parameter("ident", [65, 65], BF16, isOutput=False)
    out = nc.declare_dram_parameter("out", [S, OC], F32, isOutput=True)

    EXP = mybir.ActivationFunctionType.Exp
    ADD = mybir.AluOpType.add
    MULT = mybir.AluOpType.mult
    GE = mybir.AluOpType.is_ge

    with tile.TileContext(nc) as tc:
        with (
            tc.tile_pool(name="cst", bufs=1) as cst,
            tc.tile_pool(name="work", bufs=3) as work,
            tc.tile_pool(name="expp", bufs=8) as expp,
            tc.tile_pool(name="ps_small", bufs=2, space="PSUM") as ps_small,
            tc.tile_pool(name="ps_sc", bufs=2, space="PSUM") as ps_sc,
            tc.tile_pool(name="ps_out", bufs=2, space="PSUM") as ps_out,
        ):
            # ---- constants ----
            mask_sb = cst.tile([128, NT], F32, tag="mask")
            nc.sync.dma_start(out=mask_sb, in_=mask_t[:, :])
            ident_sb = cst.tile([65, 65], BF16, tag="ident")
            nc.sync.dma_start(out=ident_sb, in_=ident[:, :])
            bq_sb = cst.tile([128, 2], F32, tag="bq")
            nc.sync.dma_start(out=bq_sb, in_=bqp[:, :])
            bk_sb = cst.tile([128, 2], F32, tag="bk")
            nc.sync.dma_start(out=bk_sb, in_=bkp[:, :])
            # bv broadcast to all partitions: [128, 256]
            bv_sb = cst.tile([128, OC], F32, tag="bv")
            nc.gpsimd.dma_start(out=bv_sb, in_=bvf[:].partition_broadcast(128))

            # ---- big resident inputs: few large DMAs, host-packed in
            # consumption order (t4 groups 3,0,1,2; e-chunks side by side) ----
            G = {3: 0, 0: 1, 1: 2, 2: 3}  # t4 -> group position
            hT_big = cst.tile([128, 32 * 512], BF16, tag="hT_big")
            wq_big = cst.tile([128, 2048], BF16, tag="wq_big")
            nc.sync.dma_start(out=wq_big, in_=wqT[:, :])
            nc.sync.dma_start(out=hT_big[:, 0:2048], in_=hT[:, 0:2048])
            nc.sync.dma_start(out=hT_big[:, 2048:4096], in_=hT[:, 2048:4096])
            wk_big = cst.tile([128, 2048], BF16, tag="wk_big")
            nc.sync.dma_start(out=wk_big, in_=wkT[:, :])
            nc.sync.dma_start(out=hT_big[:, 4096:6144], in_=hT[:, 4096:6144])
            nc.sync.dma_start(out=hT_big[:, 6144:8192], in_=hT[:, 6144:8192])
            wv_big = cst.tile([128, 2048], BF16, tag="wv_big")
            nc.sync.dma_start(out=wv_big, in_=wvT[:, :])
            nc.sync.dma_start(out=hT_big[:, 8192:12288], in_=hT[:, 8192:12288])
            nc.sync.dma_start(out=hT_big[:, 12288:16384], in_=hT[:, 12288:16384])

            hT32 = [[hT_big[:, G[t4] * 4096 + e * 512: G[t4] * 4096 + (e + 1) * 512]
                     for t4 in range(NT4)] for e in range(8)]
            w_sb = {nm: [big[:, e * OC:(e + 1) * OC] for e in range(8)]
                    for nm, big in (("q", wq_big), ("k", wk_big), ("v", wv_big))}

            # ---- persistent intermediates ----
            qT = [cst.tile([128, S], BF16, tag=f"qT{p}", name=f"qT{p}") for p in range(NPAIR)]
            kT = [cst.tile([128, S], BF16, tag=f"kT{p}", name=f"kT{p}") for p in range(NPAIR)]
            vaug = [[cst.tile([128, 256], BF16, tag=f"va{p}_{tt}", name=f"va{p}_{tt}")
                     for tt in range(NT)] for p in range(NPAIR)]
            outsb = [cst.tile([128, OC], F32, tag=f"o{tt}", name=f"o{tt}") for tt in range(NT)]

            def emit_qk_chain(nm, p, t4):
                po = 128 * p
                dst = qT[p] if nm == "q" else kT[p]
                b_sb = bq_sb if nm == "q" else bk_sb
                ts = slice(512 * t4, 512 * (t4 + 1))
                ps_qk = ps_small.tile([128, 512], F32, tag="sm", name="ps_qk")
                for e in range(8):
                    nc.tensor.matmul(
                        ps_qk,
                        w_sb[nm][e][:, po:po + 128],
                        hT32[e][t4],
                        start=(e == 0), stop=(e == 7),
                    )
                if nm == "q":
                    nc.vector.tensor_scalar(
                        out=dst[:, ts], in0=ps_qk,
                        scalar1=0.125, scalar2=b_sb[:, p:p + 1],
                        op0=MULT, op1=ADD,
                    )
                else:
                    nc.vector.tensor_scalar_add(
                        out=dst[:, ts], in0=ps_qk, scalar1=b_sb[:, p:p + 1],
                    )

            def emit_v_chain(tt):
                t4v, r4 = divmod(tt, 4)
                rs = slice(128 * r4, 128 * (r4 + 1))
                ps_v = ps_small.tile([128, OC], F32, tag="sm", name="ps_v")
                for e in range(8):
                    nc.tensor.matmul(
                        ps_v,
                        hT32[e][t4v][:, rs],
                        w_sb["v"][e][:, :],
                        start=(e == 0), stop=(e == 7),
                    )
                for p in range(NPAIR):
                    po = 128 * p
                    vt = vaug[p][tt]
                    vt3 = vt.rearrange("a (h c) -> a h c", h=2)[:, :, 0:64]
                    ps3 = ps_v[:, po:po + 128].rearrange("a (h c) -> a h c", h=2)
                    bv3 = bv_sb[:, po:po + 128].rearrange("a (h c) -> a h c", h=2)
                    nc.vector.tensor_add(vt3, ps3, bv3)
                    nc.vector.memset(
                        vt.rearrange("a (h c) -> a h c", h=2)[:, :, 64:65], 1.0)
                    nc.vector.memset(
                        vt.rearrange("a (h c) -> a h c", h=2)[:, :, 65:128], 0.0)

            chores_q = []

            def emit_attn_block(p, j):
                po = 128 * p
                qs = slice(512 * j, 512 * (j + 1))
                out_A = ps_out.tile([128, 512], F32, tag="out", name="out_A")
                out_B = ps_out.tile([128, 512], F32, tag="out", name="out_B")
                ntk = 4 * (j + 1)
                for i in range(ntk):
                    ks = slice(128 * i, 128 * (i + 1))
                    crossing = i >= 4 * j
                    # valid tq columns of this tile start at s (cols < s are
                    # entirely above the diagonal): trim scores/exp/attn@v
                    s = 128 * i - 512 * j if crossing else 0
                    qsv = slice(512 * j + s, 512 * (j + 1))
                    sc = ps_sc.tile([128, 1024], F32, tag="sc", name="sc")
                    nc.tensor.matmul(sc[:, s:512], kT[p][0:64, ks],
                                     qT[p][0:64, qsv], start=True, stop=True)
                    nc.tensor.matmul(sc[:, 512 + s:1024], kT[p][64:128, ks],
                                     qT[p][64:128, qsv], start=True, stop=True)
                    ex = expp.tile([128, 1024], BF16, tag="exp", name="ex")
                    if s:
                        exv = ex.rearrange("a (h f) -> a h f", h=2)[:, :, s:512]
                        scv = sc.rearrange("a (h f) -> a h f", h=2)[:, :, s:512]
                    else:
                        exv, scv = ex, sc
                    nc.scalar.activation(out=exv, in_=scv, func=EXP,
                                         bias=mask_sb[:, i:i + 1], scale=1.0)
                    if crossing:
                        s_off = s
                        ex3 = ex.rearrange("a (h f) -> a h f", h=2)
                        nc.gpsimd.affine_select(
                            out=ex3, in_=ex3, compare_op=GE, fill=0.0,
                            base=-s_off, pattern=[[0, 2], [1, 512]],
                            channel_multiplier=-1,
                        )
                    nc.tensor.matmul(out_A[:, s:512], vaug[p][i][:, 0:128],
                                     ex[:, s:512],
                                     start=(i == 0), stop=(i == ntk - 1))
                    nc.tensor.matmul(out_B[:, s:512], vaug[p][i][:, 128:256],
                                     ex[:, 512 + s:1024],
                                     start=(i == 0), stop=(i == ntk - 1))
                    if chores_q:
                        chores_q.pop(0)()

                for h_loc, out_ps in ((0, out_A), (1, out_B)):
                    u = work.tile([65, 512], BF16, tag="u", name="u")
                    nc.vector.tensor_copy(u, out_ps[0:65, :])
                    for s4 in range(4):
                        tp = ps_small.tile([128, 65], BF16, tag="sm", name="tp")
                        nc.tensor.transpose(tp, u[:, 128 * s4:128 * (s4 + 1)],
                                            ident_sb)
                        r = work.tile([128, 1], F32, tag="r", name="r")
                        nc.vector.reciprocal(r, tp[:, 64:65])
                        tt = 4 * j + s4
                        c0 = po + 64 * h_loc
                        nc.vector.tensor_scalar_mul(
                            outsb[tt][:, c0:c0 + 64], tp[:, 0:64], r)
                if p == NPAIR - 1:
                    for s4 in range(4):
                        tt = 4 * j + s4
                        nc.sync.dma_start(
                            out=out[128 * tt:128 * (tt + 1), :],
                            in_=outsb[tt])

            # prologue: minimal producers for attention(p0, j=3, i=0..)
            emit_qk_chain("q", 0, 3)
            emit_qk_chain("k", 0, 0)
            for tt in range(4):
                emit_v_chain(tt)
            chores_q.append(lambda: emit_qk_chain("k", 0, 1))
            chores_q.append(lambda: emit_v_chain(4))
            chores_q.append(lambda: emit_v_chain(5))
            chores_q.append(lambda: emit_v_chain(6))
            chores_q.append(lambda: emit_qk_chain("k", 0, 2))
            chores_q.append(lambda: emit_v_chain(7))
            chores_q.append(lambda: emit_v_chain(8))
            chores_q.append(lambda: emit_v_chain(9))
            chores_q.append(lambda: emit_qk_chain("k", 0, 3))
            for _tt in range(10, 16):
                chores_q.append(lambda _tt=_tt: emit_v_chain(_tt))
            chores_q.append(lambda: emit_qk_chain("q", 0, 2))
            chores_q.append(lambda: emit_qk_chain("q", 0, 1))
            chores_q.append(lambda: emit_qk_chain("q", 0, 0))
            for t4 in (3, 2, 1, 0):
                chores_q.append(lambda t4=t4: emit_qk_chain("q", 1, t4))
                chores_q.append(lambda t4=t4: emit_qk_chain("k", 1, 3 - t4))
            # interleave the two pairs' blocks: big blocks feed ACT early,
            # small blocks finish last (short tail)
            for p, j in ((0, 3), (0, 2), (1, 3), (0, 1), (1, 2), (0, 0), (1, 1), (1, 0)):
                emit_attn_block(p, j)

    nc.compile()
    return nc


def _get_nc():
    global _cached_nc
    if _cached_nc is None:
        _cached_nc = _build()
    return _cached_nc


def make_in_maps(hidden_states, attention_mask, Wq, bq, Wk, bk, Wv, bv):
    hidden_states = np.asarray(hidden_states, dtype=np.float32)
    attention_mask = np.asarray(attention_mask, dtype=np.float32)
    Wq = np.asarray(Wq, dtype=np.float32)
    Wk = np.asarray(Wk, dtype=np.float32)
    Wv = np.asarray(Wv, dtype=np.float32)
    bq = np.asarray(bq, dtype=np.float32)
    bk = np.asarray(bk, dtype=np.float32)
    bv = np.asarray(bv, dtype=np.float32)

    bf = ml_dtypes.bfloat16
    ident = np.eye(65, dtype=np.float32).astype(bf)
    in_maps = []
    for c in range(NCORES):
        b, g = divmod(c, 4)
        cs = slice(OC * g, OC * (g + 1))
        hTT = np.ascontiguousarray(hidden_states[b].T).astype(bf)  # [E, S]
        hp = np.empty((128, 32 * 512), dtype=bf)
        for gi, t4 in enumerate((3, 0, 1, 2)):
            for e in range(8):
                hp[:, gi * 4096 + e * 512:gi * 4096 + (e + 1) * 512] = \
                    hTT[e * 128:(e + 1) * 128, t4 * 512:(t4 + 1) * 512]

        def packw(W):
            wT = np.ascontiguousarray(W[cs, :].T).astype(bf)  # [E, 256]
            wp = np.empty((128, 2048), dtype=bf)
            for e in range(8):
                wp[:, e * OC:(e + 1) * OC] = wT[e * 128:(e + 1) * 128, :]
            return wp

        in_maps.append({
            "hT": hp,
            "wqT": packw(Wq),
            "wkT": packw(Wk),
            "wvT": packw(Wv),
            "bqp": np.ascontiguousarray(bq[cs].reshape(2, 128).T),
            "bkp": np.ascontiguousarray(bk[cs].reshape(2, 128).T),
            "bvf": np.ascontiguousarray(bv[cs]),
            "mask_t": np.ascontiguousarray(
                attention_mask[b, 0, 0, :].reshape(NT, 128).T),
            "ident": ident,
        })
    return in_maps


def kernel(hidden_states, attention_mask, Wq, bq, Wk, bk, Wv, bv):
    in_maps = make_in_maps(hidden_states, attention_mask,
                           Wq, bq, Wk, bk, Wv, bv)
    nc = _get_nc()
    res = run_bass_kernel_spmd(nc, in_maps, list(range(NCORES)))

    full = np.empty((B, S, H * D), dtype=np.float32)
    for c in range(NCORES):
        b, g = divmod(c, 4)
        full[b, :, OC * g:OC * (g + 1)] = res.results[c]["out"]
    return full


# revision 12
# speedup vs baseline: 1.1524x; 1.0166x over previous
"""Causal self-attention (B=2, S=2048, E=1024, H=16, D=64) on 8 trn2 NeuronCores.

Sharding: core c = (batch b = c // 4, head-group g = c % 4).  Each core computes
4 heads (one quarter of the 16) for one batch: projections q/k/v for its 256
output channels, then causal flash-style attention, writing out[b, :, 256g:256g+256].

Per-core kernel design (Bass/Tile):
  - Host pre-transposes hidden -> hT [E, S] (bf16) and weight slices -> wT [E, 256]
    (bf16) so all matmul contractions have K on partitions.
  - q/k projections (bf16, PSUM-accumulated over 8 E-chunks) produce qT/kT in
    [d, t] layout, copied to SBUF as float32r with scale 1/8 (q) and bias add.
  - v projection produces v in [t, d] layout; DVE copy splits heads into
    v_aug tiles [tk=128, 65*2] with a ones column per head (sum-of-exp trick).
  - scores^T tiles [tk=128, tq=512] per head via single f32r matmuls; the two
    heads of a pair run concurrently on PE row halves (K=64 each).
  - exp via ScalarE activation (attention-mask bias per tk partition), bf16 out.
  - causal masking: gpsimd affine_select zeroes the invalid region of
    diagonal-crossing tiles after exp.
  - attn @ v_aug accumulates unnormalized out^T [65, tq] in PSUM (bf16 matmuls);
    row 64 is the softmax denominator.
  - PE transpose [65,128] -> [128,65], then DVE reciprocal + tensor_scalar mul
    normalizes and writes [t, d] output tiles; DMA to DRAM.
"""

import numpy as np
import ml_dtypes

import concourse.bass as bass
import concourse.mybir as mybir
import concourse.tile as tile
from concourse import bacc
from concourse.bass_utils import run_bass_kernel_spmd

F32 = mybir.dt.float32
F32R = mybir.dt.float32r
BF16 = mybir.dt.bfloat16

B, S, E = 2, 2048, 1024
H, D = 16, 64
NCORES = 8
OC = 256          # output channels per core (4 heads)
NPAIR = 2         # head pairs per core
NT = S // 128     # 16 tk tiles
NT4 = S // 512    # 4 tq blocks

_cached_nc = None


def _build():
    nc = bacc.Bacc()

    hT = nc.declare_dram_parameter("hT", [128, 32 * 512], BF16, isOutput=False)
    wqT = nc.declare_dram_parameter("wqT", [128, 2048], BF16, isOutput=False)
    wkT = nc.declare_dram_parameter("wkT", [128, 2048], BF16, isOutput=False)
    wvT = nc.declare_dram_parameter("wvT", [128, 2048], BF16, isOutput=False)
    bqp = nc.declare_dram_parameter("bqp", [128, 2], F32, isOutput=False)
    bkp = nc.declare_dram_parameter("bkp", [128, 2], F32, isOutput=False)
    bvf = nc.declare_dram_parameter("bvf", [OC], F32, isOutput=False)
    mask_t = nc.declare_dram_parameter("mask_t", [128, NT], F32, isOutput=False)
    ident = nc.declare_dram_parameter("ident", [65, 65], BF16, isOutput=False)
    out = nc.declare_dram_parameter("out", [S, OC], F32, isOutput=True)

    EXP = mybir.ActivationFunctionType.Exp
    ADD = mybir.AluOpType.add
    MULT = mybir.AluOpType.mult
    GE = mybir.AluOpType.is_ge

    with tile.TileContext(nc) as tc:
        with (
            tc.tile_pool(name="cst", bufs=1) as cst,
            tc.tile_pool(name="work", bufs=4) as work,
            tc.tile_pool(name="expp", bufs=8) as expp,
            tc.tile_pool(name="ps_small", bufs=2, space="PSUM") as ps_small,
            tc.tile_pool(name="ps_sc", bufs=2, space="PSUM") as ps_sc,
            tc.tile_pool(name="ps_out", bufs=2, space="PSUM") as ps_out,
        ):
            # ---- big resident inputs first: few large DMAs, host-packed in
            # consumption order (t4 groups 3,0,1,2; e-chunks side by side).
            # Tiny constant DMAs are deferred behind the critical prefix. ----
            G = {3: 0, 0: 1, 1: 2, 2: 3}  # t4 -> group position
            hT_big = cst.tile([128, 32 * 512], BF16, tag="hT_big")
            wq_big = cst.tile([128, 2048], BF16, tag="wq_big")
            nc.sync.dma_start(out=wq_big, in_=wqT[:, :])
            nc.sync.dma_start(out=hT_big[:, 0:2048], in_=hT[:, 0:2048])
            nc.sync.dma_start(out=hT_big[:, 2048:4096], in_=hT[:, 2048:4096])
            wk_big = cst.tile([128, 2048], BF16, tag="wk_big")
            nc.sync.dma_start(out=wk_big, in_=wkT[:, :])
            nc.sync.dma_start(out=hT_big[:, 4096:6144], in_=hT[:, 4096:6144])
            nc.sync.dma_start(out=hT_big[:, 6144:8192], in_=hT[:, 6144:8192])
            bq_sb = cst.tile([128, 2], F32, tag="bq")
            nc.sync.dma_start(out=bq_sb, in_=bqp[:, :])
            bk_sb = cst.tile([128, 2], F32, tag="bk")
            nc.sync.dma_start(out=bk_sb, in_=bkp[:, :])
            mask_sb = cst.tile([128, NT], F32, tag="mask")
            nc.sync.dma_start(out=mask_sb, in_=mask_t[:, :])
            wv_big = cst.tile([128, 2048], BF16, tag="wv_big")
            nc.sync.dma_start(out=wv_big, in_=wvT[:, :])
            bv_sb = cst.tile([128, OC], F32, tag="bv")
            nc.gpsimd.dma_start(out=bv_sb, in_=bvf[:].partition_broadcast(128))
            nc.sync.dma_start(out=hT_big[:, 8192:12288], in_=hT[:, 8192:12288])
            nc.sync.dma_start(out=hT_big[:, 12288:16384], in_=hT[:, 12288:16384])
            ident_sb = cst.tile([65, 65], BF16, tag="ident")
            nc.sync.dma_start(out=ident_sb, in_=ident[:, :])

            hT32 = [[hT_big[:, G[t4] * 4096 + e * 512: G[t4] * 4096 + (e + 1) * 512]
                     for t4 in range(NT4)] for e in range(8)]
            w_sb = {nm: [big[:, e * OC:(e + 1) * OC] for e in range(8)]
                    for nm, big in (("q", wq_big), ("k", wk_big), ("v", wv_big))}

            # ---- persistent intermediates ----
            qT = [cst.tile([128, S], BF16, tag=f"qT{p}", name=f"qT{p}") for p in range(NPAIR)]
            kT = [cst.tile([128, S], BF16, tag=f"kT{p}", name=f"kT{p}") for p in range(NPAIR)]
            vaug = [[cst.tile([128, 256], BF16, tag=f"va{p}_{tt}", name=f"va{p}_{tt}")
                     for tt in range(NT)] for p in range(NPAIR)]
            outsb = [cst.tile([128, OC], F32, tag=f"o{tt}", name=f"o{tt}") for tt in range(NT)]

            def emit_qk_chain(nm, p, t4):
                po = 128 * p
                dst = qT[p] if nm == "q" else kT[p]
                b_sb = bq_sb if nm == "q" else bk_sb
                ts = slice(512 * t4, 512 * (t4 + 1))
                ps_qk = ps_small.tile([128, 512], F32, tag="sm", name="ps_qk")
                for e in range(8):
                    nc.tensor.matmul(
                        ps_qk,
                        w_sb[nm][e][:, po:po + 128],
                        hT32[e][t4],
                        start=(e == 0), stop=(e == 7),
                    )
                if nm == "q":
                    nc.vector.tensor_scalar(
                        out=dst[:, ts], in0=ps_qk,
                        scalar1=0.125, scalar2=b_sb[:, p:p + 1],
                        op0=MULT, op1=ADD,
                    )
                else:
                    nc.vector.tensor_scalar_add(
                        out=dst[:, ts], in0=ps_qk, scalar1=b_sb[:, p:p + 1],
                    )

            def emit_v_chain(tt):
                t4v, r4 = divmod(tt, 4)
                rs = slice(128 * r4, 128 * (r4 + 1))
                ps_v = ps_small.tile([128, OC], F32, tag="sm", name="ps_v")
                for e in range(8):
                    nc.tensor.matmul(
                        ps_v,
                        hT32[e][t4v][:, rs],
                        w_sb["v"][e][:, :],
                        start=(e == 0), stop=(e == 7),
                    )
                for p in range(NPAIR):
                    po = 128 * p
                    vt = vaug[p][tt]
                    vt3 = vt.rearrange("a (h c) -> a h c", h=2)[:, :, 0:64]
                    ps3 = ps_v[:, po:po + 128].rearrange("a (h c) -> a h c", h=2)
                    bv3 = bv_sb[:, po:po + 128].rearrange("a (h c) -> a h c", h=2)
                    nc.vector.tensor_add(vt3, ps3, bv3)
                    nc.vector.memset(
                        vt.rearrange("a (h c) -> a h c", h=2)[:, :, 64:65], 1.0)
                    nc.vector.memset(
                        vt.rearrange("a (h c) -> a h c", h=2)[:, :, 65:128], 0.0)

            chores_q = []

            def emit_attn_block(p, j):
                po = 128 * p
                qs = slice(512 * j, 512 * (j + 1))
                out_A = ps_out.tile([128, 512], F32, tag="out", name="out_A")
                out_B = ps_out.tile([128, 512], F32, tag="out", name="out_B")
                ntk = 4 * (j + 1)
                for i in range(ntk):
                    ks = slice(128 * i, 128 * (i + 1))
                    crossing = i >= 4 * j
                    # valid tq columns of this tile start at s (cols < s are
                    # entirely above the diagonal): trim scores/exp/attn@v
                    s = 128 * i - 512 * j if crossing else 0
                    qsv = slice(512 * j + s, 512 * (j + 1))
                    sc = ps_sc.tile([128, 1024], F32, tag="sc", name="sc")
                    nc.tensor.matmul(sc[:, s:512], kT[p][0:64, ks],
                                     qT[p][0:64, qsv], start=True, stop=True)
                    nc.tensor.matmul(sc[:, 512 + s:1024], kT[p][64:128, ks],
                                     qT[p][64:128, qsv], start=True, stop=True)
                    ex = expp.tile([128, 1024], BF16, tag="exp", name="ex")
                    if s:
                        exv = ex.rearrange("a (h f) -> a h f", h=2)[:, :, s:512]
                        scv = sc.rearrange("a (h f) -> a h f", h=2)[:, :, s:512]
                    else:
                        exv, scv = ex, sc
                    nc.scalar.activation(out=exv, in_=scv, func=EXP,
                                         bias=mask_sb[:, i:i + 1], scale=1.0)
                    if crossing:
                        s_off = s
                        ex3 = ex.rearrange("a (h f) -> a h f", h=2)
                        nc.gpsimd.affine_select(
                            out=ex3, in_=ex3, compare_op=GE, fill=0.0,
                            base=-s_off, pattern=[[0, 2], [1, 512]],
                            channel_multiplier=-1,
                        )
                    nc.tensor.matmul(out_A[:, s:512], vaug[p][i][:, 0:128],
                                     ex[:, s:512],
                                     start=(i == 0), stop=(i == ntk - 1))
                    nc.tensor.matmul(out_B[:, s:512], vaug[p][i][:, 128:256],
                                     ex[:, 512 + s:1024],
                                     start=(i == 0), stop=(i == ntk - 1))
                    if chores_q:
                        chores_q.pop(0)()

                for h_loc, out_ps in ((0, out_A), (1, out_B)):
                    u = work.tile([65, 512], BF16, tag="u", name="u")
                    nc.vector.tensor_copy(u, out_ps[0:65, :])
                    for s4 in range(4):
                        tp = ps_small.tile([128, 65], BF16, tag="sm", name="tp")
                        nc.tensor.transpose(tp, u[:, 128 * s4:128 * (s4 + 1)],
                                            ident_sb)
                        r = work.tile([128, 1], F32, tag="r", name="r")
                        nc.vector.reciprocal(r, tp[:, 64:65])
                        tt = 4 * j + s4
                        c0 = po + 64 * h_loc
                        nc.vector.tensor_scalar_mul(
                            outsb[tt][:, c0:c0 + 64], tp[:, 0:64], r)
                if p == NPAIR - 1:
                    for s4 in range(4):
                        tt = 4 * j + s4
                        nc.sync.dma_start(
                            out=out[128 * tt:128 * (tt + 1), :],
                            in_=outsb[tt])

            # prologue: minimal producers for attention(p0, j=3, i=0..)
            emit_qk_chain("q", 0, 3)
            emit_qk_chain("k", 0, 0)
            for tt in range(4):
                emit_v_chain(tt)
            chores_q.append(lambda: emit_qk_chain("k", 0, 1))
            chores_q.append(lambda: emit_v_chain(4))
            chores_q.append(lambda: emit_v_chain(5))
            chores_q.append(lambda: emit_v_chain(6))
            chores_q.append(lambda: emit_qk_chain("k", 0, 2))
            chores_q.append(lambda: emit_v_chain(7))
            chores_q.append(lambda: emit_v_chain(8))
            chores_q.append(lambda: emit_v_chain(9))
            chores_q.append(lambda: emit_qk_chain("k", 0, 3))
            for _tt in range(10, 16):
                chores_q.append(lambda _tt=_tt: emit_v_chain(_tt))
            chores_q.append(lambda: emit_qk_chain("q", 0, 2))
            chores_q.append(lambda: emit_qk_chain("q", 0, 1))
            chores_q.append(lambda: emit_qk_chain("q", 0, 0))
            for t4 in (3, 2, 1, 0):
                chores_q.append(lambda t4=t4: emit_qk_chain("q", 1, t4))
                chores_q.append(lambda t4=t4: emit_qk_chain("k", 1, 3 - t4))
            # interleave the two pairs' blocks: big blocks feed ACT early,
            # small blocks finish last (short tail)
            for p, j in ((0, 3), (0, 2), (1, 3), (0, 1), (1, 2), (0, 0), (1, 1), (1, 0)):
                emit_attn_block(p, j)

    nc.compile()
    return nc


def _get_nc():
    global _cached_nc
    if _cached_nc is None:
        _cached_nc = _build()
    return _cached_nc


def make_in_maps(hidden_states, attention_mask, Wq, bq, Wk, bk, Wv, bv):
    hidden_states = np.asarray(hidden_states, dtype=np.float32)
    attention_mask = np.asarray(attention_mask, dtype=np.float32)
    Wq = np.asarray(Wq, dtype=np.float32)
    Wk = np.asarray(Wk, dtype=np.float32)
    Wv = np.asarray(Wv, dtype=np.float32)
    bq = np.asarray(bq, dtype=np.float32)
    bk = np.asarray(bk, dtype=np.float32)
    bv = np.asarray(bv, dtype=np.float32)

    bf = ml_dtypes.bfloat16
    ident = np.eye(65, dtype=np.float32).astype(bf)
    in_maps = []
    for c in range(NCORES):
        b, g = divmod(c, 4)
        cs = slice(OC * g, OC * (g + 1))
        hTT = np.ascontiguousarray(hidden_states[b].T).astype(bf)  # [E, S]
        hp = np.empty((128, 32 * 512), dtype=bf)
        for gi, t4 in enumerate((3, 0, 1, 2)):
            for e in range(8):
                hp[:, gi * 4096 + e * 512:gi * 4096 + (e + 1) * 512] = \
                    hTT[e * 128:(e + 1) * 128, t4 * 512:(t4 + 1) * 512]

        def packw(W):
            wT = np.ascontiguousarray(W[cs, :].T).astype(bf)  # [E, 256]
            wp = np.empty((128, 2048), dtype=bf)
            for e in range(8):
                wp[:, e * OC:(e + 1) * OC] = wT[e * 128:(e + 1) * 128, :]
            return wp

        in_maps.append({
            "hT": hp,
            "wqT": packw(Wq),
            "wkT": packw(Wk),
            "wvT": packw(Wv),
            "bqp": np.ascontiguousarray(bq[cs].reshape(2, 128).T),
            "bkp": np.ascontiguousarray(bk[cs].reshape(2, 128).T),
            "bvf": np.ascontiguousarray(bv[cs]),
            "mask_t": np.ascontiguousarray(
                attention_mask[b, 0, 0, :].reshape(NT, 128).T),
            "ident": ident,
        })
    return in_maps


def kernel(hidden_states, attention_mask, Wq, bq, Wk, bk, Wv, bv):
    in_maps = make_in_maps(hidden_states, attention_mask,
                           Wq, bq, Wk, bk, Wv, bv)
    nc = _get_nc()
    res = run_bass_kernel_spmd(nc, in_maps, list(range(NCORES)))

    full = np.empty((B, S, H * D), dtype=np.float32)
    for c in range(NCORES):
        b, g = divmod(c, 4)
        full[b, :, OC * g:OC * (g + 1)] = res.results[c]["out"]
    return full


# revision 14
# speedup vs baseline: 1.1639x; 1.0100x over previous
"""Causal self-attention (B=2, S=2048, E=1024, H=16, D=64) on 8 trn2 NeuronCores.

Sharding: core c = (batch b = c // 4, head-group g = c % 4).  Each core computes
4 heads (one quarter of the 16) for one batch: projections q/k/v for its 256
output channels, then causal flash-style attention, writing out[b, :, 256g:256g+256].

Per-core kernel design (Bass/Tile):
  - Host pre-transposes hidden -> hT [E, S] (bf16) and weight slices -> wT [E, 256]
    (bf16) so all matmul contractions have K on partitions.
  - q/k projections (bf16, PSUM-accumulated over 8 E-chunks) produce qT/kT in
    [d, t] layout, copied to SBUF as float32r with scale 1/8 (q) and bias add.
  - v projection produces v in [t, d] layout; DVE copy splits heads into
    v_aug tiles [tk=128, 65*2] with a ones column per head (sum-of-exp trick).
  - scores^T tiles [tk=128, tq=512] per head via single f32r matmuls; the two
    heads of a pair run concurrently on PE row halves (K=64 each).
  - exp via ScalarE activation (attention-mask bias per tk partition), bf16 out.
  - causal masking: gpsimd affine_select zeroes the invalid region of
    diagonal-crossing tiles after exp.
  - attn @ v_aug accumulates unnormalized out^T [65, tq] in PSUM (bf16 matmuls);
    row 64 is the softmax denominator.
  - PE transpose [65,128] -> [128,65], then DVE reciprocal + tensor_scalar mul
    normalizes and writes [t, d] output tiles; DMA to DRAM.
"""

import numpy as np
import ml_dtypes

import concourse.bass as bass
import concourse.mybir as mybir
import concourse.tile as tile
from concourse import bacc
from concourse.bass_utils import run_bass_kernel_spmd

F32 = mybir.dt.float32
F32R = mybir.dt.float32r
BF16 = mybir.dt.bfloat16

B, S, E = 2, 2048, 1024
H, D = 16, 64
NCORES = 8
OC = 256          # output channels per core (4 heads)
NPAIR = 2         # head pairs per core
NT = S // 128     # 16 tk tiles
NT4 = S // 512    # 4 tq blocks

_cached_nc = None


def _build():
    nc = bacc.Bacc()

    hT = nc.declare_dram_parameter("hT", [128, 32 * 512], BF16, isOutput=False)
    wqT = nc.declare_dram_parameter("wqT", [128, 2048], BF16, isOutput=False)
    wkT = nc.declare_dram_parameter("wkT", [128, 2048], BF16, isOutput=False)
    wvT = nc.declare_dram_parameter("wvT", [128, 2048], BF16, isOutput=False)
    bqp = nc.declare_dram_parameter("bqp", [128, 2], F32, isOutput=False)
    bkp = nc.declare_dram_parameter("bkp", [128, 2], F32, isOutput=False)
    bvf = nc.declare_dram_parameter("bvf", [OC], F32, isOutput=False)
    mask_t = nc.declare_dram_parameter("mask_t", [128, NT], F32, isOutput=False)
    ident = nc.declare_dram_parameter("ident", [65, 65], BF16, isOutput=False)
    out = nc.declare_dram_parameter("out", [S, OC], F32, isOutput=True)

    EXP = mybir.ActivationFunctionType.Exp
    ADD = mybir.AluOpType.add
    MULT = mybir.AluOpType.mult
    GE = mybir.AluOpType.is_ge

    with tile.TileContext(nc) as tc:
        with (
            tc.tile_pool(name="cst", bufs=1) as cst,
            tc.tile_pool(name="work", bufs=4) as work,
            tc.tile_pool(name="expp", bufs=8) as expp,
            tc.tile_pool(name="ps_small", bufs=2, space="PSUM") as ps_small,
            tc.tile_pool(name="ps_sc", bufs=2, space="PSUM") as ps_sc,
            tc.tile_pool(name="ps_out", bufs=2, space="PSUM") as ps_out,
        ):
            # ---- big resident inputs first: few large DMAs, host-packed in
            # consumption order (t4 groups 3,0,1,2; e-chunks side by side).
            # Tiny constant DMAs are deferred behind the critical prefix. ----
            G = {3: 0, 0: 1, 1: 2, 2: 3}  # t4 -> group position
            hT_big = cst.tile([128, 32 * 512], BF16, tag="hT_big")
            wq_big = cst.tile([128, 2048], BF16, tag="wq_big")
            nc.sync.dma_start(out=wq_big[:, 0:1024], in_=wqT[:, 0:1024])
            nc.sync.dma_start(out=hT_big[:, 0:2048], in_=hT[:, 0:2048])
            nc.sync.dma_start(out=wq_big[:, 1024:2048], in_=wqT[:, 1024:2048])
            nc.sync.dma_start(out=hT_big[:, 2048:4096], in_=hT[:, 2048:4096])
            wk_big = cst.tile([128, 2048], BF16, tag="wk_big")
            nc.sync.dma_start(out=wk_big[:, 0:1024], in_=wkT[:, 0:1024])
            nc.sync.dma_start(out=wk_big[:, 1024:2048], in_=wkT[:, 1024:2048])
            nc.sync.dma_start(out=hT_big[:, 4096:6144], in_=hT[:, 4096:6144])
            nc.sync.dma_start(out=hT_big[:, 6144:8192], in_=hT[:, 6144:8192])
            bq_sb = cst.tile([128, 2], F32, tag="bq")
            nc.sync.dma_start(out=bq_sb, in_=bqp[:, :])
            bk_sb = cst.tile([128, 2], F32, tag="bk")
            nc.sync.dma_start(out=bk_sb, in_=bkp[:, :])
            mask_sb = cst.tile([128, NT], F32, tag="mask")
            nc.sync.dma_start(out=mask_sb, in_=mask_t[:, :])
            wv_big = cst.tile([128, 2048], BF16, tag="wv_big")
            nc.sync.dma_start(out=wv_big, in_=wvT[:, :])
            bv_sb = cst.tile([128, OC], F32, tag="bv")
            nc.gpsimd.dma_start(out=bv_sb, in_=bvf[:].partition_broadcast(128))
            nc.sync.dma_start(out=hT_big[:, 8192:12288], in_=hT[:, 8192:12288])
            nc.sync.dma_start(out=hT_big[:, 12288:16384], in_=hT[:, 12288:16384])
            ident_sb = cst.tile([65, 65], BF16, tag="ident")
            nc.sync.dma_start(out=ident_sb, in_=ident[:, :])

            hT32 = [[hT_big[:, G[t4] * 4096 + e * 512: G[t4] * 4096 + (e + 1) * 512]
                     for t4 in range(NT4)] for e in range(8)]
            w_sb = {nm: [big[:, e * OC:(e + 1) * OC] for e in range(8)]
                    for nm, big in (("q", wq_big), ("k", wk_big), ("v", wv_big))}

            # ---- persistent intermediates ----
            qT = [cst.tile([128, S], BF16, tag=f"qT{p}", name=f"qT{p}") for p in range(NPAIR)]
            kT = [cst.tile([128, S], BF16, tag=f"kT{p}", name=f"kT{p}") for p in range(NPAIR)]
            vaug = [[cst.tile([128, 256], BF16, tag=f"va{p}_{tt}", name=f"va{p}_{tt}")
                     for tt in range(NT)] for p in range(NPAIR)]
            outsb_all = cst.tile([128, NT * OC], F32, tag="outsb_all", name="outsb_all")
            outsb = [outsb_all[:, OC * tt:OC * (tt + 1)] for tt in range(NT)]
            out3 = out.rearrange("(tt a) c -> tt a c", a=128)

            def emit_qk_chain(nm, p, t4):
                po = 128 * p
                dst = qT[p] if nm == "q" else kT[p]
                b_sb = bq_sb if nm == "q" else bk_sb
                ts = slice(512 * t4, 512 * (t4 + 1))
                ps_qk = ps_small.tile([128, 512], F32, tag="sm", name="ps_qk")
                for e in range(8):
                    nc.tensor.matmul(
                        ps_qk,
                        w_sb[nm][e][:, po:po + 128],
                        hT32[e][t4],
                        start=(e == 0), stop=(e == 7),
                    )
                if nm == "q":
                    nc.vector.tensor_scalar(
                        out=dst[:, ts], in0=ps_qk,
                        scalar1=0.125, scalar2=b_sb[:, p:p + 1],
                        op0=MULT, op1=ADD,
                    )
                else:
                    nc.vector.tensor_scalar_add(
                        out=dst[:, ts], in0=ps_qk, scalar1=b_sb[:, p:p + 1],
                    )

            def emit_v_chain(tt):
                t4v, r4 = divmod(tt, 4)
                rs = slice(128 * r4, 128 * (r4 + 1))
                ps_v = ps_small.tile([128, OC], F32, tag="sm", name="ps_v")
                for e in range(8):
                    nc.tensor.matmul(
                        ps_v,
                        hT32[e][t4v][:, rs],
                        w_sb["v"][e][:, :],
                        start=(e == 0), stop=(e == 7),
                    )
                for p in range(NPAIR):
                    po = 128 * p
                    vt = vaug[p][tt]
                    vt3 = vt.rearrange("a (h c) -> a h c", h=2)[:, :, 0:64]
                    ps3 = ps_v[:, po:po + 128].rearrange("a (h c) -> a h c", h=2)
                    bv3 = bv_sb[:, po:po + 128].rearrange("a (h c) -> a h c", h=2)
                    nc.vector.tensor_add(vt3, ps3, bv3)
                    nc.vector.memset(
                        vt.rearrange("a (h c) -> a h c", h=2)[:, :, 64:65], 1.0)
                    nc.vector.memset(
                        vt.rearrange("a (h c) -> a h c", h=2)[:, :, 65:128], 0.0)

            chores_q = []

            def emit_attn_block(p, j):
                po = 128 * p
                qs = slice(512 * j, 512 * (j + 1))
                out_A = ps_out.tile([128, 512], F32, tag="out", name="out_A")
                out_B = ps_out.tile([128, 512], F32, tag="out", name="out_B")
                ntk = 4 * (j + 1)
                for i in range(ntk):
                    ks = slice(128 * i, 128 * (i + 1))
                    crossing = i >= 4 * j
                    # valid tq columns of this tile start at s (cols < s are
                    # entirely above the diagonal): trim scores/exp/attn@v
                    s = 128 * i - 512 * j if crossing else 0
                    qsv = slice(512 * j + s, 512 * (j + 1))
                    sc = ps_sc.tile([128, 1024], F32, tag="sc", name="sc")
                    nc.tensor.matmul(sc[:, s:512], kT[p][0:64, ks],
                                     qT[p][0:64, qsv], start=True, stop=True)
                    nc.tensor.matmul(sc[:, 512 + s:1024], kT[p][64:128, ks],
                                     qT[p][64:128, qsv], start=True, stop=True)
                    ex = expp.tile([128, 1024], BF16, tag="exp", name="ex")
                    if s:
                        exv = ex.rearrange("a (h f) -> a h f", h=2)[:, :, s:512]
                        scv = sc.rearrange("a (h f) -> a h f", h=2)[:, :, s:512]
                    else:
                        exv, scv = ex, sc
                    nc.scalar.activation(out=exv, in_=scv, func=EXP,
                                         bias=mask_sb[:, i:i + 1], scale=1.0)
                    if crossing:
                        s_off = s
                        ex3 = ex.rearrange("a (h f) -> a h f", h=2)
                        nc.gpsimd.affine_select(
                            out=ex3, in_=ex3, compare_op=GE, fill=0.0,
                            base=-s_off, pattern=[[0, 2], [1, 512]],
                            channel_multiplier=-1,
                        )
                    nc.tensor.matmul(out_A[:, s:512], vaug[p][i][:, 0:128],
                                     ex[:, s:512],
                                     start=(i == 0), stop=(i == ntk - 1))
                    nc.tensor.matmul(out_B[:, s:512], vaug[p][i][:, 128:256],
                                     ex[:, 512 + s:1024],
                                     start=(i == 0), stop=(i == ntk - 1))
                    if chores_q:
                        chores_q.pop(0)()

                for h_loc, out_ps in ((0, out_A), (1, out_B)):
                    u = work.tile([65, 512], BF16, tag="u", name="u")
                    nc.vector.tensor_copy(u, out_ps[0:65, :])
                    for s4 in range(4):
                        tp = ps_small.tile([128, 65], BF16, tag="sm", name="tp")
                        nc.tensor.transpose(tp, u[:, 128 * s4:128 * (s4 + 1)],
                                            ident_sb)
                        r = work.tile([128, 1], F32, tag="r", name="r")
                        nc.vector.reciprocal(r, tp[:, 64:65])
                        tt = 4 * j + s4
                        c0 = po + 64 * h_loc
                        nc.vector.tensor_scalar_mul(
                            outsb[tt][:, c0:c0 + 64], tp[:, 0:64], r)
                if p == NPAIR - 1:
                    for s4 in range(4):
                        tt = 4 * j + s4
                        nc.sync.dma_start(out=out3[tt, :, :], in_=outsb[tt])

            # prologue: minimal producers for attention(p0, j=3, i=0..)
            emit_qk_chain("q", 0, 3)
            emit_qk_chain("k", 0, 0)
            for tt in range(4):
                emit_v_chain(tt)
            chores_q.append(lambda: emit_qk_chain("k", 0, 1))
            chores_q.append(lambda: emit_v_chain(4))
            chores_q.append(lambda: emit_v_chain(5))
            chores_q.append(lambda: emit_v_chain(6))
            chores_q.append(lambda: emit_qk_chain("k", 0, 2))
            chores_q.append(lambda: emit_v_chain(7))
            chores_q.append(lambda: emit_v_chain(8))
            chores_q.append(lambda: emit_v_chain(9))
            chores_q.append(lambda: emit_qk_chain("k", 0, 3))
            for _tt in range(10, 16):
                chores_q.append(lambda _tt=_tt: emit_v_chain(_tt))
            chores_q.append(lambda: emit_qk_chain("q", 0, 2))
            chores_q.append(lambda: emit_qk_chain("q", 0, 1))
            chores_q.append(lambda: emit_qk_chain("q", 0, 0))
            for t4 in (3, 2, 1, 0):
                chores_q.append(lambda t4=t4: emit_qk_chain("q", 1, t4))
                chores_q.append(lambda t4=t4: emit_qk_chain("k", 1, 3 - t4))
            # interleave the two pairs' blocks: big blocks feed ACT early,
            # small blocks finish last (short tail)
            for p, j in ((0, 3), (0, 2), (1, 3), (0, 1), (1, 2), (0, 0), (1, 1), (1, 0)):
                emit_attn_block(p, j)

    nc.compile()
    return nc


def _get_nc():
    global _cached_nc
    if _cached_nc is None:
        _cached_nc = _build()
    return _cached_nc


def make_in_maps(hidden_states, attention_mask, Wq, bq, Wk, bk, Wv, bv):
    hidden_states = np.asarray(hidden_states, dtype=np.float32)
    attention_mask = np.asarray(attention_mask, dtype=np.float32)
    Wq = np.asarray(Wq, dtype=np.float32)
    Wk = np.asarray(Wk, dtype=np.float32)
    Wv = np.asarray(Wv, dtype=np.float32)
    bq = np.asarray(bq, dtype=np.float32)
    bk = np.asarray(bk, dtype=np.float32)
    bv = np.asarray(bv, dtype=np.float32)

    bf = ml_dtypes.bfloat16
    ident = np.eye(65, dtype=np.float32).astype(bf)
    in_maps = []
    for c in range(NCORES):
        b, g = divmod(c, 4)
        cs = slice(OC * g, OC * (g + 1))
        hTT = np.ascontiguousarray(hidden_states[b].T).astype(bf)  # [E, S]
        hp = np.empty((128, 32 * 512), dtype=bf)
        for gi, t4 in enumerate((3, 0, 1, 2)):
            for e in range(8):
                hp[:, gi * 4096 + e * 512:gi * 4096 + (e + 1) * 512] = \
                    hTT[e * 128:(e + 1) * 128, t4 * 512:(t4 + 1) * 512]

        def packw(W):
            wT = np.ascontiguousarray(W[cs, :].T).astype(bf)  # [E, 256]
            wp = np.empty((128, 2048), dtype=bf)
            for e in range(8):
                wp[:, e * OC:(e + 1) * OC] = wT[e * 128:(e + 1) * 128, :]
            return wp

        in_maps.append({
            "hT": hp,
            "wqT": packw(Wq),
            "wkT": packw(Wk),
            "wvT": packw(Wv),
            "bqp": np.ascontiguousarray(bq[cs].reshape(2, 128).T),
            "bkp": np.ascontiguousarray(bk[cs].reshape(2, 128).T),
            "bvf": np.ascontiguousarray(bv[cs]),
            "mask_t": np.ascontiguousarray(
                attention_mask[b, 0, 0, :].reshape(NT, 128).T),
            "ident": ident,
        })
    return in_maps


def kernel(hidden_states, attention_mask, Wq, bq, Wk, bk, Wv, bv):
    in_maps = make_in_maps(hidden_states, attention_mask,
                           Wq, bq, Wk, bk, Wv, bv)
    nc = _get_nc()
    res = run_bass_kernel_spmd(nc, in_maps, list(range(NCORES)))

    full = np.empty((B, S, H * D), dtype=np.float32)
    for c in range(NCORES):
        b, g = divmod(c, 4)
        full[b, :, OC * g:OC * (g + 1)] = res.results[c]["out"]
    return full
